# revision 7
# baseline (speedup 1.0000x reference)
"""Two-layer GAT (GATConv x2, PyG-style self-loops) on 8 Trainium2 cores.

Strategy (dst-major slots, batched dma_gather):
  - Nodes are permuted host-side: greedy 4-coloring balances each dst's
    in-edge sources across classes (class = pid % 4, needed because
    dma_gather indices are int16: idx = src_pid // 4 <= 25088); nodes are
    then sorted by class-count vector so 128-node dst blocks are
    degree- and class-homogeneous, which minimizes slot padding.
  - Per dst slot: edge slots [128 dst x K], gathered with one dma_gather
    call per class (<=8 k-columns per call), thousands of descriptors per
    call instead of one SWDGE launch per 128 edges.
  - Node table rows: layer-1 [h(128) | u(2) v(2) f(2) g(2)] = 136 floats
    (gather elem 192 floats, %256B), layer-2 [h2(64) | u2 v2] = 66 floats
    (elem 128), where u=e^{a_s}, v=e^{0.2 a_s}, f=e^{a_d}, g=e^{0.2 a_d}:
    exp(leaky_relu(a_s+a_d)) == max(u*f, v*g) exactly, so per-edge softmax
    weights are broadcast mults and a max; storing u2/v2 removes the
    per-edge layer-2 attention dot entirely.
  - Per slot, both heads fused per DVE op: uf/vg [P,K,2] mults, w = max,
    one strided reduce for den, one 3D-broadcast mult for w*h, one strided
    reduce for the numerator. Self-loop and bias fold into
    scalar_tensor_tensor epilogues; copies/exps run on the Scalar engine.
  - p1 computes the core's own shard table (98 matmuls), AllGathers it;
    the layer-2 node transform (ELU -> transpose -> @W2cat -> row) is fused
    into e1's epilogue, then a second AllGather shares the layer-2 table.

kernel() takes full inputs, returns the full [100000, 64] output (fp32
everywhere: the rel-err metric's 1e-3 floor leaves ~2e-5 abs budget).
"""
import sys
from contextlib import ExitStack

import numpy as np

# ---------------- problem constants (hardcoded per harness contract) -------
N = 100000
NCORES = 8
P = 128
F_IN = 128
H1 = 2
C1 = 64
HC1 = 128
C2 = 64
NS = 12544          # nodes per core shard = 98 * 128
NSLOT = NS // P     # 98 dst slots per core
NPAD = NS * NCORES  # 100352
NBLK = NPAD // P    # 784
W1R = 144           # layer-1 row: h(128) u(2) v(2) f(2) g(2) pad(8)
GW1 = 4 * W1R       # 576 floats per 4-row group (%64 floats)
EW1 = 192           # gather elem floats (>=136, %64)
W2R = 80            # layer-2 row: h2(64) u2 v2 pad(14)
GW2 = 4 * W2R       # 320 (%64)
EW2 = 128           # >=66, %64
GRP = NS // 4 + 2   # groups per core shard incl. 2 zero pad groups
NG = NCORES * GRP   # full table groups (AllGather of 8 shards)
NEG_SLOPE = 0.2


def _ensure_axon_hooks():
    """bass_utils' trace path needs antenv.axon_hooks; provide it if absent."""
    import types
    try:
        import antenv.axon_hooks as mod
    except ImportError:
        import antenv
        mod = types.ModuleType("antenv.axon_hooks")
        mod._hook = None
        def set_axon_ntff_profile_hook(hook):
            mod._hook = hook
        def get_axon_ntff_profile_hook():
            return mod._hook
        mod.set_axon_ntff_profile_hook = set_axon_ntff_profile_hook
        mod.get_axon_ntff_profile_hook = get_axon_ntff_profile_hook
        sys.modules["antenv.axon_hooks"] = mod
        antenv.axon_hooks = mod
    if mod.get_axon_ntff_profile_hook() is None:
        try:
            from trn_agent_boot.trn_boot import _ntff_profile_via_ctypes
            hook = _ntff_profile_via_ctypes("/opt/axon/libaxon_pjrt.so")
            if hook is not None:
                mod.set_axon_ntff_profile_hook(hook)
        except Exception:
            pass


# ---------------- host-side graph preprocessing ----------------------------
def _color_and_permute(src, dst):
    """Greedy 4-coloring (balance each dst's sources across classes) and a
    node permutation: pid % 4 == class, blocks sorted by class-count vector.
    Returns perm (node -> pid)."""
    odeg = np.bincount(src, minlength=NPAD)
    proc = np.argsort(-odeg, kind="stable")
    es = np.argsort(src, kind="stable")
    ss, dd = src[es], dst[es]
    starts = np.searchsorted(ss, np.arange(NPAD + 1))
    cnt = np.zeros((NPAD, 4), np.int32)     # per-dst class counts
    quota = np.full(4, NPAD // 4, np.int64)
    color = np.full(NPAD, -1, np.int8)
    has_out = odeg > 0
    for n in proc:
        if not has_out[n]:
            break  # proc is sorted by out-degree desc
        a, b = starts[n], starts[n + 1]
        nb = dd[a:b]
        score = cnt[nb].sum(0).astype(np.float64)
        score += (1.0 - quota / (NPAD // 4)) * 0.5
        score[quota <= 0] = np.inf
        c = int(np.argmin(score))
        color[n] = c
        quota[c] -= 1
        cnt[nb, c] += 1
    # refinement: move a node's color where it most reduces its dsts'
    # class-count imbalance
    for _ in range(5):
        for n in np.flatnonzero(has_out):
            a, b = starts[n], starts[n + 1]
            nb = dd[a:b]
            c1 = color[n]
            s = cnt[nb].sum(0)
            c2 = int(np.argmin(s))
            if c2 != c1 and s[c2] + len(nb) < s[c1]:
                cnt[nb, c1] -= 1
                cnt[nb, c2] += 1
                color[n] = c2
    sizes = np.bincount(color[color >= 0], minlength=4)
    while sizes.max() > NPAD // 4:
        c1 = int(sizes.argmax())
        cand = np.flatnonzero(color == c1)
        n = cand[np.argmin(odeg[cand])]
        c2 = int(sizes.argmin())
        a, b = starts[n], starts[n + 1]
        cnt[dd[a:b], c1] -= 1
        cnt[dd[a:b], c2] += 1
        color[n] = c2
        sizes[c1] -= 1
        sizes[c2] += 1
    left = np.flatnonzero(color < 0)
    fill = np.repeat(np.arange(4), (NPAD // 4 - sizes).clip(0))[:len(left)]
    color[left] = fill
    assert (np.bincount(color, minlength=4) == NPAD // 4).all()
    key = np.lexsort((cnt[:, 3], cnt[:, 2], cnt[:, 1], cnt[:, 0]))
    # sorted block rank r -> (core r%8, slot r//8); within a block position
    # p takes color p%4 (pid%4 == p%4).
    queues = [key[color[key] == c] for c in range(4)]
    pos = [0, 0, 0, 0]
    perm = np.empty(NPAD, np.int64)   # node -> pid
    for g in range(NPAD):
        r, p = g // P, g % P
        c = p % 4
        n = queues[c][pos[c]]
        pos[c] += 1
        perm[n] = (r % NCORES) * NS + (r // NCORES) * P + p
    return perm


def _build_slot_tables(src, dst, perm):
    """Per-core, per-slot, per-class edge slot tables in pid space.
    K_sched[s][c] = max over the 8 cores of the block max per-partition
    class count. Returns K_sched [98][4] and per-core int16 idx streams."""
    psrc = perm[src]
    pdst = perm[dst]
    blk = pdst // P
    part = pdst % P
    cls = psrc % 4
    cnt = np.zeros((NPAD, 4), np.int32)
    np.add.at(cnt, (pdst, cls), 1)
    bmax = cnt.reshape(NBLK, P, 4).max(1)             # [784, 4]
    # pid block b: core b % 8 (rank r=b? no: pid = (r%8)*NS + (r//8)*128+p)
    # => block index in pid space: b = core*NSLOT + slot
    K_sched = bmax.reshape(NBLK // NSLOT, NSLOT, 4).max(0)  # wrong axis fix below
    # recompute correctly: pid block b: core b // NSLOT, slot b % NSLOT
    K_sched = bmax.reshape(NCORES, NSLOT, 4).max(0)   # [98, 4]
    ktot = K_sched.sum(1)

    order = np.lexsort((part, cls, blk))
    pb, pc, pp, ps = blk[order], cls[order], part[order], psrc[order]
    key = (pb * 4 + pc) * P + pp
    counts = np.bincount(key, minlength=NBLK * 4 * P)
    kpos = np.arange(len(order)) - np.repeat(
        np.concatenate([[0], np.cumsum(counts)[:-1]]), counts)

    # idx value = table group row: shards AllGathered with 2 zero pad groups
    # per core appended, so group g lands at row g + 2*(g//(NS//4));
    # pad slots point at core 0's zero rows (row NS//4).
    PADG = NS // 4
    koff = np.zeros((NSLOT, 4), np.int64)
    koff[:, 1:] = np.cumsum(K_sched, 1)[:, :-1]
    soff = np.concatenate([[0], np.cumsum(ktot)])
    total_cols = int(soff[-1])
    idx = np.full((NCORES, total_cols * P), PADG, np.int32)
    core = pb // NSLOT
    slot = pb % NSLOT
    pos_in_stream = (soff[slot] + koff[slot, pc] + kpos) * P + pp
    grp = ps // 4
    idx[core, pos_in_stream] = grp + 2 * (grp // (NS // 4))
    # wrap in 16 partitions: position i -> [i%16, i//16]; replicate 8x.
    idx16 = np.ascontiguousarray(
        idx.reshape(NCORES, total_cols * P // 16, 16).transpose(0, 2, 1)
    ).astype(np.int16)
    idx16 = np.ascontiguousarray(np.tile(idx16, (1, 8, 1)))
    return K_sched, idx16, soff


def _att_cat(W, att_src, att_dst):
    h, c = att_src.shape
    cin = W.shape[1]
    As = np.zeros((cin, h), np.float32)
    Ad = np.zeros((cin, h), np.float32)
    for i in range(h):
        As[i * c:(i + 1) * c, i] = att_src[i]
        Ad[i * c:(i + 1) * c, i] = att_dst[i]
    return np.concatenate([W, W @ As, W @ Ad], 1).astype(np.float32)


# ---------------- bass program --------------------------------------------
def _build_program(K_sched, soff):
    import concourse.bass as bass
    import concourse.tile as tile
    from concourse import mybir, library_config
    from concourse.library_overlay import lower_extended_insts
    from concourse.vector_clock import ScopedClock

    f32 = mybir.dt.float32
    i16 = mybir.dt.int16
    Act = mybir.ActivationFunctionType
    Alu = mybir.AluOpType
    X = mybir.AxisListType.X

    total_cols = int(soff[-1])
    kmax = int(max(K_sched.sum(1)))

    class PatchedTileContext(tile.TileContext):
        """Kernel-tail drain must not carry more waits than the ISA allows;
        split them across chained drains (this walrus allows 1 wait/inst)."""
        def _drain_and_barrier(self, tick_clock, wait_clock):
            drain_inst = self.nc.sync.drain()
            wait_clock.add_sem_waits(
                drain_inst.ins, ScopedClock({None: tick_clock.global_clock})
            )
            si = drain_inst.ins.sync_info
            if si is not None and si.on_wait and len(si.on_wait) > 1:
                waits = list(si.on_wait)
                si.on_wait = waits[:1]
                rest = waits[1:]
                while rest:
                    extra = self.nc.sync.drain()
                    extra.ins.sync_info = mybir.SyncInfo(on_wait=rest[:1], on_update=[])
                    rest = rest[1:]
            self.nc.all_engine_barrier()
            assert self.sems is not None
            popped = self.nc._tile_sem_poison_stack.pop()
            assert popped is self._sem_poison
            self.nc.clear_and_free_semaphores(list(self.sems.allocated().values()))
            self.nc.all_engine_barrier()

    nc = bass.Bass(num_devices=NCORES, num_swdge_queues=4)

    xTs = nc.declare_dram_parameter("xTs", [P, NS], f32, isOutput=False)
    w1cat = nc.declare_dram_parameter("w1cat", [P, HC1 + 2 * H1], f32, isOutput=False)
    w2cat = nc.declare_dram_parameter("w2cat", [P, C2 + 2], f32, isOutput=False)
    b1row = nc.declare_dram_parameter("b1row", [1, HC1], f32, isOutput=False)
    b2row = nc.declare_dram_parameter("b2row", [1, C2], f32, isOutput=False)
    ident_in = nc.declare_dram_parameter("ident", [P, P], f32, isOutput=False)
    idx_in = nc.declare_dram_parameter("idx16", [P, total_cols * P // 16],
                                       i16, isOutput=False)
    padg1 = nc.declare_dram_parameter("padg1", [2, GW1], f32, isOutput=False)
    padg2 = nc.declare_dram_parameter("padg2", [2, GW2], f32, isOutput=False)
    out2 = nc.declare_dram_parameter("out2", [NS, C2], f32, isOutput=True)

    with PatchedTileContext(nc) as tc, ExitStack() as ctx:
        nc.gpsimd.load_library(library_config.mlp)
        const = ctx.enter_context(tc.tile_pool(name="const", bufs=1))
        dram = ctx.enter_context(tc.tile_pool(name="dram", bufs=1, space="DRAM"))

        tab1s = dram.tile([GRP, GW1], f32)                     # own shard L1
        tab1 = dram.tile([NG, GW1], f32, addr_space="Shared")  # full L1
        tab2s = dram.tile([GRP, GW2], f32)
        tab2 = dram.tile([NG, GW2], f32, addr_space="Shared")

        w1_sb = const.tile([P, HC1 + 2 * H1], f32)
        nc.sync.dma_start(out=w1_sb[:], in_=w1cat[:])
        w2_sb = const.tile([P, C2 + 2], f32)
        nc.sync.dma_start(out=w2_sb[:], in_=w2cat[:])
        b1_sb = const.tile([P, HC1], f32)
        nc.sync.dma_start(out=b1_sb[:], in_=b1row[0:1, :].to_broadcast([P, HC1]))
        b2_sb = const.tile([P, C2], f32)
        nc.sync.dma_start(out=b2_sb[:], in_=b2row[0:1, :].to_broadcast([P, C2]))
        ident_sb = const.tile([P, P], f32)
        nc.sync.dma_start(out=ident_sb[:], in_=ident_in[:])
        idx_sb = const.tile([P, total_cols * P // 16], i16)
        nc.sync.dma_start(out=idx_sb[:], in_=idx_in[:])
        uvfg2 = const.tile([P, NSLOT * 2], f32)   # per-slot [f2, g2]
        # zero pad groups at each shard's tail (u=v=0 -> pad slots add 0)
        nc.sync.dma_start(out=tab1s[GRP - 2:GRP, :], in_=padg1[:])
        nc.sync.dma_start(out=tab2s[GRP - 2:GRP, :], in_=padg2[:])

        # ---- p1: own-shard node transform -> tab1s ----
        with nc.named_scope("p1"), ExitStack() as c2:
            sbp = c2.enter_context(tc.tile_pool(name="p1sb", bufs=3))
            psp = c2.enter_context(tc.tile_pool(name="p1ps", bufs=3, space="PSUM"))
            for sl in range(14):
                slab = sbp.tile([P, 7 * P], f32, tag="slab")
                nc.sync.dma_start(out=slab[:], in_=xTs[:, sl * 896:(sl + 1) * 896])
                for k in range(7):
                    s = sl * 7 + k
                    ps = psp.tile([P, HC1 + 2 * H1], f32, tag="ps")
                    nc.tensor.matmul(out=ps[:], lhsT=slab[:, k * P:(k + 1) * P],
                                     rhs=w1_sb[:], start=True, stop=True)
                    t1 = sbp.tile([P, W1R], f32, tag="t1")
                    nc.scalar.copy(out=t1[:, 0:HC1], in_=ps[:, 0:HC1])
                    nc.scalar.activation(out=t1[:, 128:130],
                                         in_=ps[:, 128:130], func=Act.Exp)
                    nc.scalar.activation(out=t1[:, 130:132],
                                         in_=ps[:, 128:130], func=Act.Exp,
                                         scale=NEG_SLOPE)
                    nc.scalar.activation(out=t1[:, 132:134],
                                         in_=ps[:, 130:132], func=Act.Exp)
                    nc.scalar.activation(out=t1[:, 134:136],
                                         in_=ps[:, 130:132], func=Act.Exp,
                                         scale=NEG_SLOPE)
                    nc.vector.memset(t1[:, 136:W1R], 0.0)
                    nc.sync.dma_start(
                        out=tab1s[s * 32:(s + 1) * 32, :].rearrange(
                            "g (r w) -> (g r) w", r=4),
                        in_=t1[:])

        # ---- AllGather layer-1 table ----
        nc.gpsimd.collective_compute(
            "AllGather", mybir.AluOpType.bypass,
            replica_groups=[list(range(NCORES))],
            ins=[tab1s[:, :].opt()],
            outs=[tab1[:, :].opt()],
        )

        reg_cache = {}
        qctr = [0]

        def nreg(n):
            if n not in reg_cache:
                reg_cache[n] = nc.gpsimd.to_reg(n)
            return reg_cache[n]

        def issue_gather(pool, s, tab, gw, ew, w_row):
            ktot = int(K_sched[s].sum())
            g = pool.tile([P, kmax * ew], f32, tag="g")
            co = 0
            for c in range(4):
                kc = int(K_sched[s][c])
                k0 = 0
                while k0 < kc:
                    kch = min(kc - k0, 8)   # <=1024 descriptors per call
                    n_idx = P * kch
                    ioff = (int(soff[s]) + co) * P // 16
                    in_ap = bass.AP(tab[:, :].tensor, c * w_row,
                                    [[gw, NG - 1], [1, ew]])
                    nc.gpsimd.dma_gather(
                        out_ap=g[:, co * ew:(co + kch) * ew].rearrange(
                            "p (k e) -> p k e", k=kch),
                        in_ap=in_ap,
                        idxs_ap=idx_sb[:, ioff:ioff + n_idx // 16],
                        num_idxs=n_idx, num_idxs_reg=nreg(n_idx),
                        elem_size=ew, elem_step=gw,
                        queue_num=qctr[0] % 4,
                    )
                    qctr[0] += 1
                    co += kch
                    k0 += kch
            return g, ktot

        # ---- e1: layer-1 edge phase (emits layer-2 table rows) ----
        with nc.named_scope("e1"), ExitStack() as c2:
            sbg = c2.enter_context(tc.tile_pool(name="e1g", bufs=3))
            sbo = c2.enter_context(tc.tile_pool(name="e1o", bufs=4))
            sbm = c2.enter_context(tc.tile_pool(name="e1m", bufs=2))
            sbs = c2.enter_context(tc.tile_pool(name="e1s", bufs=3))
            psp = c2.enter_context(tc.tile_pool(name="e1ps", bufs=2, space="PSUM"))

            def issue1(s):
                g, ktot = issue_gather(sbg, s, tab1, GW1, EW1, W1R)
                ow = sbo.tile([P, W1R], f32, tag="ow")
                nc.sync.dma_start(
                    out=ow[:],
                    in_=tab1s[s * 32:(s + 1) * 32, :].rearrange(
                        "g (r w) -> (g r) w", r=4))
                return g, ktot, ow

            PF = 3
            pre = [issue1(s) for s in range(PF)]
            for s in range(NSLOT):
                g, K, ow = pre.pop(0)
                if s + PF < NSLOT:
                    pre.append(issue1(s + PF))
                gv = g[:, 0:K * EW1].rearrange("p (k w) -> p k w", w=EW1)
                # per-edge uf/vg for both heads: [P, K, 2]
                uf2 = sbs.tile([P, 2 * kmax], f32, tag="uf2")
                nc.vector.tensor_tensor(
                    out=uf2[:, 0:2 * K].rearrange("p (k h) -> p k h", h=2),
                    in0=gv[:, :, 128:130],
                    in1=ow[:, 132:134].rearrange("p h -> p () h").to_broadcast(
                        [P, K, 2]),
                    op=Alu.mult)
                vg2 = sbs.tile([P, 2 * kmax], f32, tag="vg2")
                nc.vector.tensor_tensor(
                    out=vg2[:, 0:2 * K].rearrange("p (k h) -> p k h", h=2),
                    in0=gv[:, :, 130:132],
                    in1=ow[:, 134:136].rearrange("p h -> p () h").to_broadcast(
                        [P, K, 2]),
                    op=Alu.mult)
                # self-loop weight per head
                sa = sbs.tile([P, 2], f32, tag="sa")
                nc.vector.tensor_tensor(out=sa[:], in0=ow[:, 128:130],
                                        in1=ow[:, 132:134], op=Alu.mult)
                sb = sbs.tile([P, 2], f32, tag="sb")
                nc.vector.tensor_tensor(out=sb[:], in0=ow[:, 130:132],
                                        in1=ow[:, 134:136], op=Alu.mult)
                selfw = sbs.tile([P, 2], f32, tag="selfw")
                nc.vector.tensor_tensor(out=selfw[:], in0=sa[:], in1=sb[:],
                                        op=Alu.max)
                # w = max(uf, vg); den = self_w + sum_k w
                wt2 = sbs.tile([P, 2 * kmax], f32, tag="wt2")
                nc.vector.tensor_tensor(out=wt2[:, 0:2 * K], in0=uf2[:, 0:2 * K],
                                        in1=vg2[:, 0:2 * K], op=Alu.max)
                den0 = sbs.tile([P, 2], f32, tag="den0")
                nc.vector.tensor_reduce(
                    out=den0[:],
                    in_=wt2[:, 0:2 * K].rearrange("p (k h) -> p h k", h=2),
                    axis=X, op=Alu.add)
                den = sbs.tile([P, 2], f32, tag="den")
                nc.vector.tensor_tensor(out=den[:], in0=den0[:], in1=selfw[:],
                                        op=Alu.add)
                # messages and numerator (both heads in one op each)
                msgs = sbm.tile([P, kmax * HC1], f32, tag="msgs")
                nc.vector.tensor_tensor(
                    out=msgs[:, 0:K * HC1].rearrange(
                        "p (k h c) -> p k h c", h=2, c=C1),
                    in0=gv[:, :, 0:HC1].rearrange(
                        "p k (h c) -> p k h c", c=C1),
                    in1=wt2[:, 0:2 * K].rearrange(
                        "p (k h) -> p k h ()", h=2).to_broadcast([P, K, 2, C1]),
                    op=Alu.mult)
                numr = sbs.tile([P, HC1], f32, tag="numr")
                nc.vector.tensor_reduce(
                    out=numr[:],
                    in_=msgs[:, 0:K * HC1].rearrange("p (k c) -> p c k", c=HC1),
                    axis=X, op=Alu.add)
                rec = sbs.tile([P, 2], f32, tag="rec")
                nc.vector.reciprocal(out=rec[:], in_=den[:])
                ob = sbs.tile([P, HC1], f32, tag="ob")
                for h in range(2):
                    cs = slice(h * C1, (h + 1) * C1)
                    num3 = sbs.tile([P, C1], f32, tag=f"num3{h}")
                    nc.vector.scalar_tensor_tensor(
                        out=num3[:], in0=ow[:, cs], scalar=selfw[:, h:h + 1],
                        in1=numr[:, cs], op0=Alu.mult, op1=Alu.add)
                    nc.vector.scalar_tensor_tensor(
                        out=ob[:, cs], in0=num3[:], scalar=rec[:, h:h + 1],
                        in1=b1_sb[:, cs], op0=Alu.mult, op1=Alu.add)
                # ELU -> transpose -> @W2cat -> layer-2 table row
                negm = sbs.tile([P, HC1], f32, tag="negm")
                nc.vector.tensor_scalar_min(out=negm[:], in0=ob[:], scalar1=0.0)
                pos = sbs.tile([P, HC1], f32, tag="pos")
                nc.scalar.activation(out=pos[:], in_=ob[:], func=Act.Relu)
                em = sbs.tile([P, HC1], f32, tag="em")
                nc.scalar.activation(out=em[:], in_=negm[:], func=Act.Exp)
                h1b = sbs.tile([P, HC1], f32, tag="h1b")
                nc.vector.scalar_tensor_tensor(
                    out=h1b[:], in0=em[:], scalar=-1.0, in1=pos[:],
                    op0=Alu.add, op1=Alu.add)
                ps_t = psp.tile([P, P], f32, tag="pst")
                nc.tensor.transpose(out=ps_t[:], in_=h1b[:], identity=ident_sb[:])
                h1t = sbs.tile([P, P], f32, tag="h1t")
                nc.scalar.copy(out=h1t[:], in_=ps_t[:])
                ps2 = psp.tile([P, C2 + 2], f32, tag="ps2")
                nc.tensor.matmul(out=ps2[:], lhsT=h1t[:], rhs=w2_sb[:],
                                 start=True, stop=True)
                t2 = sbs.tile([P, W2R], f32, tag="t2")
                nc.scalar.copy(out=t2[:, 0:C2], in_=ps2[:, 0:C2])
                nc.scalar.activation(out=t2[:, 64:65], in_=ps2[:, C2:C2 + 1],
                                     func=Act.Exp)
                nc.scalar.activation(out=t2[:, 65:66], in_=ps2[:, C2:C2 + 1],
                                     func=Act.Exp, scale=NEG_SLOPE)
                nc.scalar.activation(out=uvfg2[:, 2 * s:2 * s + 1],
                                     in_=ps2[:, C2 + 1:C2 + 2], func=Act.Exp)
                nc.scalar.activation(out=uvfg2[:, 2 * s + 1:2 * s + 2],
                                     in_=ps2[:, C2 + 1:C2 + 2], func=Act.Exp,
                                     scale=NEG_SLOPE)
                nc.vector.memset(t2[:, 66:W2R], 0.0)
                nc.sync.dma_start(
                    out=tab2s[s * 32:(s + 1) * 32, :].rearrange(
                        "g (r w) -> (g r) w", r=4),
                    in_=t2[:])

        # ---- AllGather layer-2 table ----
        nc.gpsimd.collective_compute(
            "AllGather", mybir.AluOpType.bypass,
            replica_groups=[list(range(NCORES))],
            ins=[tab2s[:, :].opt()],
            outs=[tab2[:, :].opt()],
        )

        # ---- e2: layer-2 edge phase -> out2 ----
        with nc.named_scope("e2"), ExitStack() as c2:
            sbg = c2.enter_context(tc.tile_pool(name="e2g", bufs=3))
            sbo = c2.enter_context(tc.tile_pool(name="e2o", bufs=4))
            sbm = c2.enter_context(tc.tile_pool(name="e2m", bufs=2))
            sbs = c2.enter_context(tc.tile_pool(name="e2s", bufs=3))

            def issue2(s):
                g, ktot = issue_gather(sbg, s, tab2, GW2, EW2, W2R)
                ow = sbo.tile([P, W2R], f32, tag="ow")
                nc.sync.dma_start(
                    out=ow[:],
                    in_=tab2s[s * 32:(s + 1) * 32, :].rearrange(
                        "g (r w) -> (g r) w", r=4))
                return g, ktot, ow

            PF = 3
            pre = [issue2(s) for s in range(PF)]
            for s in range(NSLOT):
                g, K, ow = pre.pop(0)
                if s + PF < NSLOT:
                    pre.append(issue2(s + PF))
                gv = g[:, 0:K * EW2].rearrange("p (k w) -> p k w", w=EW2)
                uf = sbs.tile([P, kmax], f32, tag="uf")
                nc.vector.tensor_scalar_mul(
                    out=uf[:, 0:K], in0=gv[:, :, 64],
                    scalar1=uvfg2[:, 2 * s:2 * s + 1])
                vg = sbs.tile([P, kmax], f32, tag="vg")
                nc.vector.tensor_scalar_mul(
                    out=vg[:, 0:K], in0=gv[:, :, 65],
                    scalar1=uvfg2[:, 2 * s + 1:2 * s + 2])
                sa = sbs.tile([P, 1], f32, tag="sa")
                nc.vector.tensor_tensor(out=sa[:], in0=ow[:, 64:65],
                                        in1=uvfg2[:, 2 * s:2 * s + 1], op=Alu.mult)
                sb = sbs.tile([P, 1], f32, tag="sb")
                nc.vector.tensor_tensor(out=sb[:], in0=ow[:, 65:66],
                                        in1=uvfg2[:, 2 * s + 1:2 * s + 2],
                                        op=Alu.mult)
                selfw = sbs.tile([P, 1], f32, tag="selfw")
                nc.vector.tensor_tensor(out=selfw[:], in0=sa[:], in1=sb[:],
                                        op=Alu.max)
                wt = sbs.tile([P, kmax], f32, tag="wt")
                nc.vector.tensor_tensor(out=wt[:, 0:K], in0=uf[:, 0:K],
                                        in1=vg[:, 0:K], op=Alu.max)
                den0 = sbs.tile([P, 1], f32, tag="den0")
                nc.vector.tensor_reduce(out=den0[:], in_=wt[:, 0:K],
                                        axis=X, op=Alu.add)
                den = sbs.tile([P, 1], f32, tag="den")
                nc.vector.tensor_tensor(out=den[:], in0=den0[:], in1=selfw[:],
                                        op=Alu.add)
                msgs = sbm.tile([P, kmax * C2], f32, tag="msgs")
                nc.vector.tensor_tensor(
                    out=msgs[:, 0:K * C2].rearrange("p (k c) -> p k c", c=C2),
                    in0=gv[:, :, 0:C2],
                    in1=wt[:, 0:K].rearrange("p k -> p k ()").to_broadcast(
                        [P, K, C2]),
                    op=Alu.mult)
                numr = sbs.tile([P, C2], f32, tag="numr")
                nc.vector.tensor_reduce(
                    out=numr[:],
                    in_=msgs[:, 0:K * C2].rearrange("p (k c) -> p c k", c=C2),
                    axis=X, op=Alu.add)
                rec = sbs.tile([P, 1], f32, tag="rec")
                nc.vector.reciprocal(out=rec[:], in_=den[:])
                num3 = sbs.tile([P, C2], f32, tag="num3")
                nc.vector.scalar_tensor_tensor(
                    out=num3[:], in0=ow[:, 0:C2], scalar=selfw[:, 0:1],
                    in1=numr[:], op0=Alu.mult, op1=Alu.add)
                ob = sbs.tile([P, C2], f32, tag="ob")
                nc.vector.scalar_tensor_tensor(
                    out=ob[:], in0=num3[:], scalar=rec[:, 0:1],
                    in1=b2_sb[:], op0=Alu.mult, op1=Alu.add)
                nc.sync.dma_start(out=out2[s * P:(s + 1) * P, :], in_=ob[:])

    _split_overloaded_waits(nc)
    lower_extended_insts(nc)
    return nc


def _split_overloaded_waits(nc):
    """This walrus build accepts one sem wait per instruction; hoist extras
    onto NoOps spliced immediately before (same engine => same ordering)."""
    from concourse import mybir
    n_fix = 0
    for bb in nc.main_func.blocks:
        insts = bb.instructions
        out = []
        for ins in insts:
            si = getattr(ins, "sync_info", None)
            waits = list(si.on_wait) if (si and si.on_wait) else []
            if len(waits) > 1:
                si.on_wait = waits[-1:]
                rest = waits[:-1]
                while rest:
                    nop = mybir.InstNoOp(name=f"wsplit-{nc.next_id()}", ins=[], outs=[])
                    nop.engine = ins.engine
                    nop.sync_info = mybir.SyncInfo(on_wait=rest[:1], on_update=[])
                    rest = rest[1:]
                    out.append(nop)
                n_fix += 1
            out.append(ins)
        if len(out) != len(insts):
            insts.clear()
            insts.extend(out)
    return n_fix


# ---------------- entry point ----------------------------------------------
_LAST_EXEC_NS = None
_LAST_SCOPES = None


def kernel(x, edge_index, W1, att_src1, att_dst1, b1, W2, att_src2, att_dst2,
           b2, _trace=False):
    global _LAST_EXEC_NS, _LAST_SCOPES
    W1m, W2m = W1, W2
    _ensure_axon_hooks()
    import concourse.bass_utils as bass_utils
    bass_utils.upload_artifacts = lambda tmpdir: tmpdir
    from concourse.bass_utils import run_bass_kernel_spmd

    x = np.asarray(x, np.float32)
    src = np.asarray(edge_index[0], np.int64)
    dst = np.asarray(edge_index[1], np.int64)

    perm = _color_and_permute(src, dst)
    K_sched, idx16, soff = _build_slot_tables(src, dst, perm)

    w1c = _att_cat(np.asarray(W1m, np.float32),
                   np.asarray(att_src1, np.float32),
                   np.asarray(att_dst1, np.float32))
    w2c = _att_cat(np.asarray(W2m, np.float32),
                   np.asarray(att_src2, np.float32),
                   np.asarray(att_dst2, np.float32))
    b1r = np.asarray(b1, np.float32).reshape(1, HC1)
    b2r = np.asarray(b2, np.float32).reshape(1, C2)

    inv = np.empty(NPAD, np.int64)   # pid -> node
    inv[perm] = np.arange(NPAD)
    xp = np.zeros((NPAD, F_IN), np.float32)
    real = inv < N
    xp[real] = x[inv[real]]
    ident = np.eye(P, dtype=np.float32)

    nc = _build_program(K_sched, soff)
    in_maps = []
    for d in range(NCORES):
        xTs = np.ascontiguousarray(xp[d * NS:(d + 1) * NS].T)
        in_maps.append(dict(
            xTs=xTs, w1cat=w1c, w2cat=w2c, b1row=b1r, b2row=b2r,
            ident=ident, idx16=np.ascontiguousarray(idx16[d]),
            padg1=np.zeros((2, GW1), np.float32),
            padg2=np.zeros((2, GW2), np.float32),
        ))
    res = run_bass_kernel_spmd(nc, in_maps, list(range(NCORES)), trace=_trace)
    _LAST_EXEC_NS = res.exec_time_ns
    _LAST_SCOPES = res.per_core_scope_times
    outp = np.concatenate([res.results[d]["out2"] for d in range(NCORES)], 0)
    out = np.empty((N, C2), np.float32)
    out[:] = outp[perm[:N]]
    return out


# revision 11
# speedup vs baseline: 1.7898x; 1.7898x over previous
"""Two-layer GAT (GATConv x2, PyG-style self-loops) on 8 Trainium2 cores.

Strategy (dst-major slots, batched dma_gather):
  - Nodes are permuted host-side: greedy 4-coloring balances each dst's
    in-edge sources across classes (class = pid % 4, needed because
    dma_gather indices are int16: idx = src_pid // 4 <= 25088); nodes are
    then sorted by class-count vector so 128-node dst blocks are
    degree- and class-homogeneous, which minimizes slot padding.
  - Per dst slot: edge slots [128 dst x K], gathered with one dma_gather
    call per class (<=8 k-columns per call), thousands of descriptors per
    call instead of one SWDGE launch per 128 edges.
  - Node table rows: layer-1 [h(128) | u(2) v(2) f(2) g(2)] = 136 floats
    (gather elem 192 floats, %256B), layer-2 [h2(64) | u2 v2] = 66 floats
    (elem 128), where u=e^{a_s}, v=e^{0.2 a_s}, f=e^{a_d}, g=e^{0.2 a_d}:
    exp(leaky_relu(a_s+a_d)) == max(u*f, v*g) exactly, so per-edge softmax
    weights are broadcast mults and a max; storing u2/v2 removes the
    per-edge layer-2 attention dot entirely.
  - Per slot, both heads fused per DVE op: uf/vg [P,K,2] mults, w = max,
    one strided reduce for den, one 3D-broadcast mult for w*h, one strided
    reduce for the numerator. Self-loop and bias fold into
    scalar_tensor_tensor epilogues; copies/exps run on the Scalar engine.
  - p1 computes the core's own shard table (98 matmuls), AllGathers it;
    the layer-2 node transform (ELU -> transpose -> @W2cat -> row) is fused
    into e1's epilogue, then a second AllGather shares the layer-2 table.

kernel() takes full inputs, returns the full [100000, 64] output (fp32
everywhere: the rel-err metric's 1e-3 floor leaves ~2e-5 abs budget).
"""
import sys
from contextlib import ExitStack

import numpy as np

# ---------------- problem constants (hardcoded per harness contract) -------
N = 100000
NCORES = 8
P = 128
F_IN = 128
H1 = 2
C1 = 64
HC1 = 128
C2 = 64
NS = 12544          # nodes per core shard = 98 * 128
NSLOT = NS // P     # 98 dst slots per core
NPAD = NS * NCORES  # 100352
NBLK = NPAD // P    # 784
W1R = 144           # layer-1 row: h(128) u(2) v(2) f(2) g(2) pad(8)
GW1 = 4 * W1R       # 576 floats per 4-row group (%64 floats)
EW1 = 192           # gather elem floats (>=136, %64)
W2R = 64            # layer-2 row: h2 only (u2/v2 computed on-chip)
GW2 = 4 * W2R       # 256 (%64)
EW2 = 64            # %64
GRP = NS // 4 + 2   # groups per core shard incl. 2 zero pad groups
NG = NCORES * GRP   # full table groups (AllGather of 8 shards)
NEG_SLOPE = 0.2


def _ensure_axon_hooks():
    """bass_utils' trace path needs antenv.axon_hooks; provide it if absent."""
    import types
    try:
        import antenv.axon_hooks as mod
    except ImportError:
        import antenv
        mod = types.ModuleType("antenv.axon_hooks")
        mod._hook = None
        def set_axon_ntff_profile_hook(hook):
            mod._hook = hook
        def get_axon_ntff_profile_hook():
            return mod._hook
        mod.set_axon_ntff_profile_hook = set_axon_ntff_profile_hook
        mod.get_axon_ntff_profile_hook = get_axon_ntff_profile_hook
        sys.modules["antenv.axon_hooks"] = mod
        antenv.axon_hooks = mod
    if mod.get_axon_ntff_profile_hook() is None:
        try:
            from trn_agent_boot.trn_boot import _ntff_profile_via_ctypes
            hook = _ntff_profile_via_ctypes("/opt/axon/libaxon_pjrt.so")
            if hook is not None:
                mod.set_axon_ntff_profile_hook(hook)
        except Exception:
            pass


# ---------------- host-side graph preprocessing ----------------------------
def _color_and_permute(src, dst):
    """Greedy 4-coloring (balance each dst's sources across classes) and a
    node permutation: pid % 4 == class, blocks sorted by class-count vector.
    Returns perm (node -> pid)."""
    odeg = np.bincount(src, minlength=NPAD)
    proc = np.argsort(-odeg, kind="stable")
    es = np.argsort(src, kind="stable")
    ss, dd = src[es], dst[es]
    starts = np.searchsorted(ss, np.arange(NPAD + 1))
    cnt = np.zeros((NPAD, 4), np.int32)     # per-dst class counts
    quota = np.full(4, NPAD // 4, np.int64)
    color = np.full(NPAD, -1, np.int8)
    has_out = odeg > 0
    for n in proc:
        if not has_out[n]:
            break  # proc is sorted by out-degree desc
        a, b = starts[n], starts[n + 1]
        nb = dd[a:b]
        score = cnt[nb].sum(0).astype(np.float64)
        score += (1.0 - quota / (NPAD // 4)) * 0.5
        score[quota <= 0] = np.inf
        c = int(np.argmin(score))
        color[n] = c
        quota[c] -= 1
        cnt[nb, c] += 1
    # refinement: move a node's color where it most reduces its dsts'
    # class-count imbalance
    for _ in range(5):
        for n in np.flatnonzero(has_out):
            a, b = starts[n], starts[n + 1]
            nb = dd[a:b]
            c1 = color[n]
            s = cnt[nb].sum(0)
            c2 = int(np.argmin(s))
            if c2 != c1 and s[c2] + len(nb) < s[c1]:
                cnt[nb, c1] -= 1
                cnt[nb, c2] += 1
                color[n] = c2
    sizes = np.bincount(color[color >= 0], minlength=4)
    while sizes.max() > NPAD // 4:
        c1 = int(sizes.argmax())
        cand = np.flatnonzero(color == c1)
        n = cand[np.argmin(odeg[cand])]
        c2 = int(sizes.argmin())
        a, b = starts[n], starts[n + 1]
        cnt[dd[a:b], c1] -= 1
        cnt[dd[a:b], c2] += 1
        color[n] = c2
        sizes[c1] -= 1
        sizes[c2] += 1
    left = np.flatnonzero(color < 0)
    fill = np.repeat(np.arange(4), (NPAD // 4 - sizes).clip(0))[:len(left)]
    color[left] = fill
    assert (np.bincount(color, minlength=4) == NPAD // 4).all()
    mx = cnt.max(1)
    am = cnt.argmax(1)
    key = np.lexsort((cnt[:, 3], cnt[:, 2], cnt[:, 1], cnt[:, 0], am, mx))
    # sorted block rank r -> (core r%8, slot r//8); within a block position
    # p takes color p%4 (pid%4 == p%4).
    queues = [key[color[key] == c] for c in range(4)]
    pos = [0, 0, 0, 0]
    perm = np.empty(NPAD, np.int64)   # node -> pid
    for g in range(NPAD):
        r, p = g // P, g % P
        c = p % 4
        n = queues[c][pos[c]]
        pos[c] += 1
        perm[n] = (r % NCORES) * NS + (r // NCORES) * P + p
    return perm


def _build_slot_tables(src, dst, perm):
    """Per-core, per-slot, per-class edge slot tables in pid space.
    K_sched[s][c] = max over the 8 cores of the block max per-partition
    class count. Returns K_sched [98][4] and per-core int16 idx streams."""
    psrc = perm[src]
    pdst = perm[dst]
    blk = pdst // P
    part = pdst % P
    cls = psrc % 4
    cnt = np.zeros((NPAD, 4), np.int32)
    np.add.at(cnt, (pdst, cls), 1)
    bmax = cnt.reshape(NBLK, P, 4).max(1)             # [784, 4]
    # pid block b: core b % 8 (rank r=b? no: pid = (r%8)*NS + (r//8)*128+p)
    # => block index in pid space: b = core*NSLOT + slot
    K_sched = bmax.reshape(NBLK // NSLOT, NSLOT, 4).max(0)  # wrong axis fix below
    # recompute correctly: pid block b: core b // NSLOT, slot b % NSLOT
    K_sched = bmax.reshape(NCORES, NSLOT, 4).max(0)   # [98, 4]
    ktot = K_sched.sum(1)

    order = np.lexsort((part, cls, blk))
    pb, pc, pp, ps = blk[order], cls[order], part[order], psrc[order]
    key = (pb * 4 + pc) * P + pp
    counts = np.bincount(key, minlength=NBLK * 4 * P)
    kpos = np.arange(len(order)) - np.repeat(
        np.concatenate([[0], np.cumsum(counts)[:-1]]), counts)

    # idx value = table group row: shards AllGathered with 2 zero pad groups
    # per core appended, so group g lands at row g + 2*(g//(NS//4));
    # pad slots point at core 0's zero rows (row NS//4).
    PADG = NS // 4
    koff = np.zeros((NSLOT, 4), np.int64)
    koff[:, 1:] = np.cumsum(K_sched, 1)[:, :-1]
    soff = np.concatenate([[0], np.cumsum(ktot)])
    total_cols = int(soff[-1])
    idx = np.full((NCORES, total_cols * P), PADG, np.int32)
    core = pb // NSLOT
    slot = pb % NSLOT
    pos_in_stream = (soff[slot] + koff[slot, pc] + kpos) * P + pp
    grp = ps // 4
    idx[core, pos_in_stream] = grp + 2 * (grp // (NS // 4))
    # wrap in 16 partitions: position i -> [i%16, i//16]; replicate 8x.
    idx16 = np.ascontiguousarray(
        idx.reshape(NCORES, total_cols * P // 16, 16).transpose(0, 2, 1)
    ).astype(np.int16)
    idx16 = np.ascontiguousarray(np.tile(idx16, (1, 8, 1)))
    return K_sched, idx16, soff


def _att_cat(W, att_src, att_dst):
    h, c = att_src.shape
    cin = W.shape[1]
    As = np.zeros((cin, h), np.float32)
    Ad = np.zeros((cin, h), np.float32)
    for i in range(h):
        As[i * c:(i + 1) * c, i] = att_src[i]
        Ad[i * c:(i + 1) * c, i] = att_dst[i]
    return np.concatenate([W, W @ As, W @ Ad], 1).astype(np.float32)


# ---------------- bass program --------------------------------------------
def _build_program(K_sched, soff):
    import concourse.bass as bass
    import concourse.tile as tile
    from concourse import mybir, library_config
    from concourse.library_overlay import lower_extended_insts
    from concourse.vector_clock import ScopedClock

    f32 = mybir.dt.float32
    i16 = mybir.dt.int16
    Act = mybir.ActivationFunctionType
    Alu = mybir.AluOpType
    X = mybir.AxisListType.X

    total_cols = int(soff[-1])
    kmax = int(max(K_sched.sum(1)))

    class PatchedTileContext(tile.TileContext):
        """Kernel-tail drain must not carry more waits than the ISA allows;
        split them across chained drains (this walrus allows 1 wait/inst)."""
        def _drain_and_barrier(self, tick_clock, wait_clock):
            drain_inst = self.nc.sync.drain()
            wait_clock.add_sem_waits(
                drain_inst.ins, ScopedClock({None: tick_clock.global_clock})
            )
            si = drain_inst.ins.sync_info
            if si is not None and si.on_wait and len(si.on_wait) > 1:
                waits = list(si.on_wait)
                si.on_wait = waits[:1]
                rest = waits[1:]
                while rest:
                    extra = self.nc.sync.drain()
                    extra.ins.sync_info = mybir.SyncInfo(on_wait=rest[:1], on_update=[])
                    rest = rest[1:]
            self.nc.all_engine_barrier()
            assert self.sems is not None
            popped = self.nc._tile_sem_poison_stack.pop()
            assert popped is self._sem_poison
            self.nc.clear_and_free_semaphores(list(self.sems.allocated().values()))
            self.nc.all_engine_barrier()

    nc = bass.Bass(num_devices=NCORES, num_swdge_queues=4)

    xTs = nc.declare_dram_parameter("xTs", [P, NS], f32, isOutput=False)
    w1cat = nc.declare_dram_parameter("w1cat", [P, HC1 + 2 * H1], f32, isOutput=False)
    w2cat = nc.declare_dram_parameter("w2cat", [P, C2 + 2], f32, isOutput=False)
    b1row = nc.declare_dram_parameter("b1row", [1, HC1], f32, isOutput=False)
    b2row = nc.declare_dram_parameter("b2row", [1, C2], f32, isOutput=False)
    as2row = nc.declare_dram_parameter("as2row", [1, C2], f32, isOutput=False)
    ident_in = nc.declare_dram_parameter("ident", [P, P], f32, isOutput=False)
    idx_in = nc.declare_dram_parameter("idx16", [P, total_cols * P // 16],
                                       i16, isOutput=False)
    padg1 = nc.declare_dram_parameter("padg1", [2, GW1], f32, isOutput=False)
    padg2 = nc.declare_dram_parameter("padg2", [2, GW2], f32, isOutput=False)
    out2 = nc.declare_dram_parameter("out2", [NS, C2], f32, isOutput=True)

    with PatchedTileContext(nc) as tc, ExitStack() as ctx:
        nc.gpsimd.load_library(library_config.mlp)
        const = ctx.enter_context(tc.tile_pool(name="const", bufs=1))
        dram = ctx.enter_context(tc.tile_pool(name="dram", bufs=1, space="DRAM"))

        tab1s = dram.tile([GRP, GW1], f32)                     # own shard L1
        tab1 = dram.tile([NG, GW1], f32, addr_space="Shared")  # full L1
        tab2s = dram.tile([GRP, GW2], f32)
        tab2 = dram.tile([NG, GW2], f32, addr_space="Shared")

        w1_sb = const.tile([P, HC1 + 2 * H1], f32)
        nc.sync.dma_start(out=w1_sb[:], in_=w1cat[:])
        w2_sb = const.tile([P, C2 + 2], f32)
        nc.sync.dma_start(out=w2_sb[:], in_=w2cat[:])
        b1_sb = const.tile([P, HC1], f32)
        nc.sync.dma_start(out=b1_sb[:], in_=b1row[0:1, :].to_broadcast([P, HC1]))
        b2_sb = const.tile([P, C2], f32)
        nc.sync.dma_start(out=b2_sb[:], in_=b2row[0:1, :].to_broadcast([P, C2]))
        as2_sb = const.tile([P, C2], f32)
        nc.sync.dma_start(out=as2_sb[:], in_=as2row[0:1, :].to_broadcast([P, C2]))
        ident_sb = const.tile([P, P], f32)
        nc.sync.dma_start(out=ident_sb[:], in_=ident_in[:])
        idx_sb = const.tile([P, total_cols * P // 16], i16)
        nc.sync.dma_start(out=idx_sb[:], in_=idx_in[:])
        uvfg2 = const.tile([P, NSLOT * 3], f32)   # per-slot [selfw2, f2, g2]
        sw1 = const.tile([P, NSLOT * 2], f32)     # per-node layer-1 self weight
        # zero pad groups at each shard's tail (u=v=0 -> pad slots add 0)
        nc.sync.dma_start(out=tab1s[GRP - 2:GRP, :], in_=padg1[:])
        nc.sync.dma_start(out=tab2s[GRP - 2:GRP, :], in_=padg2[:])

        # ---- p1: own-shard node transform -> tab1s ----
        with nc.named_scope("p1"), ExitStack() as c2:
            sbp = c2.enter_context(tc.tile_pool(name="p1sb", bufs=3))
            psp = c2.enter_context(tc.tile_pool(name="p1ps", bufs=3, space="PSUM"))
            for sl in range(14):
                slab = sbp.tile([P, 7 * P], f32, tag="slab")
                nc.sync.dma_start(out=slab[:], in_=xTs[:, sl * 896:(sl + 1) * 896])
                for k in range(7):
                    s = sl * 7 + k
                    ps = psp.tile([P, HC1 + 2 * H1], f32, tag="ps")
                    nc.tensor.matmul(out=ps[:], lhsT=slab[:, k * P:(k + 1) * P],
                                     rhs=w1_sb[:], start=True, stop=True)
                    t1 = sbp.tile([P, W1R], f32, tag="t1")
                    nc.scalar.copy(out=t1[:, 0:HC1], in_=ps[:, 0:HC1])
                    nc.scalar.activation(out=t1[:, 128:130],
                                         in_=ps[:, 128:130], func=Act.Exp)
                    nc.scalar.activation(out=t1[:, 130:132],
                                         in_=ps[:, 128:130], func=Act.Exp,
                                         scale=NEG_SLOPE)
                    nc.scalar.activation(out=t1[:, 132:134],
                                         in_=ps[:, 130:132], func=Act.Exp)
                    nc.scalar.activation(out=t1[:, 134:136],
                                         in_=ps[:, 130:132], func=Act.Exp,
                                         scale=NEG_SLOPE)
                    nc.vector.memset(t1[:, 136:W1R], 0.0)
                    sa1 = sbp.tile([P, 4], f32, tag="sa1")
                    nc.vector.tensor_tensor(out=sa1[:, 0:2], in0=t1[:, 128:130],
                                            in1=t1[:, 132:134], op=Alu.mult)
                    nc.vector.tensor_tensor(out=sa1[:, 2:4], in0=t1[:, 130:132],
                                            in1=t1[:, 134:136], op=Alu.mult)
                    nc.vector.tensor_tensor(out=sw1[:, 2 * s:2 * s + 2],
                                            in0=sa1[:, 0:2], in1=sa1[:, 2:4],
                                            op=Alu.max)
                    nc.sync.dma_start(
                        out=tab1s[s * 32:(s + 1) * 32, :].rearrange(
                            "g (r w) -> (g r) w", r=4),
                        in_=t1[:])

        # ---- AllGather layer-1 table ----
        nc.gpsimd.collective_compute(
            "AllGather", mybir.AluOpType.bypass,
            replica_groups=[list(range(NCORES))],
            ins=[tab1s[:, :].opt()],
            outs=[tab1[:, :].opt()],
        )

        reg_cache = {}
        qctr = [0]

        def nreg(n):
            if n not in reg_cache:
                reg_cache[n] = nc.gpsimd.to_reg(n)
            return reg_cache[n]

        def issue_gather(pool, s, tab, gw, ew, w_row):
            ktot = int(K_sched[s].sum())
            g = pool.tile([P, kmax * ew], f32, tag="g")
            co = 0
            for c in range(4):
                kc = int(K_sched[s][c])
                k0 = 0
                while k0 < kc:
                    kch = min(kc - k0, 8)   # <=1024 descriptors per call
                    n_idx = P * kch
                    ioff = (int(soff[s]) + co) * P // 16
                    in_ap = bass.AP(tab[:, :].tensor, c * w_row,
                                    [[gw, NG - 1], [1, ew]])
                    nc.gpsimd.dma_gather(
                        out_ap=g[:, co * ew:(co + kch) * ew].rearrange(
                            "p (k e) -> p k e", k=kch),
                        in_ap=in_ap,
                        idxs_ap=idx_sb[:, ioff:ioff + n_idx // 16],
                        num_idxs=n_idx, num_idxs_reg=nreg(n_idx),
                        elem_size=ew, elem_step=gw,
                        queue_num=qctr[0] % 4,
                    )
                    qctr[0] += 1
                    co += kch
                    k0 += kch
            return g, ktot

        # ---- e1: layer-1 edge phase (emits layer-2 table rows) ----
        with nc.named_scope("e1"), ExitStack() as c2:
            sbg = c2.enter_context(tc.tile_pool(name="e1g", bufs=3))
            sbo = c2.enter_context(tc.tile_pool(name="e1o", bufs=4))
            sbm = c2.enter_context(tc.tile_pool(name="e1m", bufs=2))
            sbs = c2.enter_context(tc.tile_pool(name="e1s", bufs=3))
            psp = c2.enter_context(tc.tile_pool(name="e1ps", bufs=2, space="PSUM"))

            def issue1(s):
                g, ktot = issue_gather(sbg, s, tab1, GW1, EW1, W1R)
                ow = sbo.tile([P, W1R], f32, tag="ow")
                nc.sync.dma_start(
                    out=ow[:],
                    in_=tab1s[s * 32:(s + 1) * 32, :].rearrange(
                        "g (r w) -> (g r) w", r=4))
                return g, ktot, ow

            PF = 3
            pre = [issue1(s) for s in range(PF)]
            for s in range(NSLOT):
                g, K, ow = pre.pop(0)
                if s + PF < NSLOT:
                    pre.append(issue1(s + PF))
                gv = g[:, 0:K * EW1].rearrange("p (k w) -> p k w", w=EW1)
                # per-edge uf/vg for both heads: [P, K, 2]
                uf2 = sbs.tile([P, 2 * kmax], f32, tag="uf2")
                nc.vector.tensor_tensor(
                    out=uf2[:, 0:2 * K].rearrange("p (k h) -> p k h", h=2),
                    in0=gv[:, :, 128:130],
                    in1=ow[:, 132:134].rearrange("p h -> p () h").to_broadcast(
                        [P, K, 2]),
                    op=Alu.mult)
                vg2 = sbs.tile([P, 2 * kmax], f32, tag="vg2")
                nc.vector.tensor_tensor(
                    out=vg2[:, 0:2 * K].rearrange("p (k h) -> p k h", h=2),
                    in0=gv[:, :, 130:132],
                    in1=ow[:, 134:136].rearrange("p h -> p () h").to_broadcast(
                        [P, K, 2]),
                    op=Alu.mult)
                selfw = sw1[:, 2 * s:2 * s + 2]
                # w = max(uf, vg); den = self_w + sum_k w
                wt2 = sbs.tile([P, 2 * kmax], f32, tag="wt2")
                nc.vector.tensor_tensor(out=wt2[:, 0:2 * K], in0=uf2[:, 0:2 * K],
                                        in1=vg2[:, 0:2 * K], op=Alu.max)
                den0 = sbs.tile([P, 2], f32, tag="den0")
                nc.vector.tensor_reduce(
                    out=den0[:],
                    in_=wt2[:, 0:2 * K].rearrange("p (k h) -> p h k", h=2),
                    axis=X, op=Alu.add)
                den = sbs.tile([P, 2], f32, tag="den")
                nc.vector.tensor_tensor(out=den[:], in0=den0[:], in1=selfw,
                                        op=Alu.add)
                # messages and numerator (both heads in one op each)
                msgs = sbm.tile([P, kmax * HC1], f32, tag="msgs")
                nc.vector.tensor_tensor(
                    out=msgs[:, 0:K * HC1].rearrange(
                        "p (k h c) -> p k h c", h=2, c=C1),
                    in0=gv[:, :, 0:HC1].rearrange(
                        "p k (h c) -> p k h c", c=C1),
                    in1=wt2[:, 0:2 * K].rearrange(
                        "p (k h) -> p k h ()", h=2).to_broadcast([P, K, 2, C1]),
                    op=Alu.mult)
                numr = sbs.tile([P, HC1], f32, tag="numr")
                nc.vector.tensor_reduce(
                    out=numr[:],
                    in_=msgs[:, 0:K * HC1].rearrange("p (k c) -> p c k", c=HC1),
                    axis=X, op=Alu.add)
                rec = sbs.tile([P, 2], f32, tag="rec")
                nc.vector.reciprocal(out=rec[:], in_=den[:])
                ob = sbs.tile([P, HC1], f32, tag="ob")
                for h in range(2):
                    cs = slice(h * C1, (h + 1) * C1)
                    num3 = sbs.tile([P, C1], f32, tag=f"num3{h}")
                    nc.vector.scalar_tensor_tensor(
                        out=num3[:], in0=ow[:, cs], scalar=sw1[:, 2 * s + h:2 * s + h + 1],
                        in1=numr[:, cs], op0=Alu.mult, op1=Alu.add)
                    nc.vector.scalar_tensor_tensor(
                        out=ob[:, cs], in0=num3[:], scalar=rec[:, h:h + 1],
                        in1=b1_sb[:, cs], op0=Alu.mult, op1=Alu.add)
                # ELU -> transpose -> @W2cat -> layer-2 table row
                negm = sbs.tile([P, HC1], f32, tag="negm")
                nc.vector.tensor_scalar_min(out=negm[:], in0=ob[:], scalar1=0.0)
                pos = sbs.tile([P, HC1], f32, tag="pos")
                nc.scalar.activation(out=pos[:], in_=ob[:], func=Act.Relu)
                em = sbs.tile([P, HC1], f32, tag="em")
                nc.scalar.activation(out=em[:], in_=negm[:], func=Act.Exp)
                h1b = sbs.tile([P, HC1], f32, tag="h1b")
                nc.vector.scalar_tensor_tensor(
                    out=h1b[:], in0=em[:], scalar=-1.0, in1=pos[:],
                    op0=Alu.add, op1=Alu.add)
                ps_t = psp.tile([P, P], f32, tag="pst")
                nc.tensor.transpose(out=ps_t[:], in_=h1b[:], identity=ident_sb[:])
                h1t = sbs.tile([P, P], f32, tag="h1t")
                nc.scalar.copy(out=h1t[:], in_=ps_t[:])
                ps2 = psp.tile([P, C2 + 2], f32, tag="ps2")
                nc.tensor.matmul(out=ps2[:], lhsT=h1t[:], rhs=w2_sb[:],
                                 start=True, stop=True)
                t2 = sbs.tile([P, W2R], f32, tag="t2")
                nc.scalar.copy(out=t2[:, 0:C2], in_=ps2[:, 0:C2])
                nc.scalar.activation(out=uvfg2[:, 3 * s + 1:3 * s + 2],
                                     in_=ps2[:, C2 + 1:C2 + 2], func=Act.Exp)
                nc.scalar.activation(out=uvfg2[:, 3 * s + 2:3 * s + 3],
                                     in_=ps2[:, C2 + 1:C2 + 2], func=Act.Exp,
                                     scale=NEG_SLOPE)
                se2 = sbs.tile([P, 4], f32, tag="se2")
                nc.scalar.activation(out=se2[:, 0:1], in_=ps2[:, C2:C2 + 1],
                                     func=Act.Exp)
                nc.scalar.activation(out=se2[:, 1:2], in_=ps2[:, C2:C2 + 1],
                                     func=Act.Exp, scale=NEG_SLOPE)
                sm2 = sbs.tile([P, 2], f32, tag="sm2")
                nc.vector.tensor_tensor(out=sm2[:, 0:1], in0=se2[:, 0:1],
                                        in1=uvfg2[:, 3 * s + 1:3 * s + 2],
                                        op=Alu.mult)
                nc.vector.tensor_tensor(out=sm2[:, 1:2], in0=se2[:, 1:2],
                                        in1=uvfg2[:, 3 * s + 2:3 * s + 3],
                                        op=Alu.mult)
                nc.vector.tensor_tensor(out=uvfg2[:, 3 * s:3 * s + 1],
                                        in0=sm2[:, 0:1], in1=sm2[:, 1:2],
                                        op=Alu.max)
                nc.sync.dma_start(
                    out=tab2s[s * 32:(s + 1) * 32, :].rearrange(
                        "g (r w) -> (g r) w", r=4),
                    in_=t2[:])

        # ---- AllGather layer-2 table ----
        nc.gpsimd.collective_compute(
            "AllGather", mybir.AluOpType.bypass,
            replica_groups=[list(range(NCORES))],
            ins=[tab2s[:, :].opt()],
            outs=[tab2[:, :].opt()],
        )

        # ---- e2: layer-2 edge phase -> out2 ----
        with nc.named_scope("e2"), ExitStack() as c2:
            sbg = c2.enter_context(tc.tile_pool(name="e2g", bufs=3))
            sbo = c2.enter_context(tc.tile_pool(name="e2o", bufs=4))
            sbm = c2.enter_context(tc.tile_pool(name="e2m", bufs=2))
            sbs = c2.enter_context(tc.tile_pool(name="e2s", bufs=3))

            def issue2(s):
                g, ktot = issue_gather(sbg, s, tab2, GW2, EW2, W2R)
                ow = sbo.tile([P, W2R], f32, tag="ow")
                nc.sync.dma_start(
                    out=ow[:],
                    in_=tab2s[s * 32:(s + 1) * 32, :].rearrange(
                        "g (r w) -> (g r) w", r=4))
                return g, ktot, ow

            PF = 6
            pre = [issue2(s) for s in range(PF)]
            for s in range(NSLOT):
                g, K, ow = pre.pop(0)
                if s + PF < NSLOT:
                    pre.append(issue2(s + PF))
                gv = g[:, 0:K * EW2].rearrange("p (k w) -> p k w", w=EW2)
                # per-edge a_s2 dot, then u2/v2 via exp
                prod = sbm.tile([P, kmax * C2], f32, tag="prod")
                nc.vector.tensor_tensor(
                    out=prod[:, 0:K * C2].rearrange("p (k c) -> p k c", c=C2),
                    in0=gv[:, :, 0:C2],
                    in1=as2_sb[:].rearrange("p c -> p () c").to_broadcast(
                        [P, K, C2]),
                    op=Alu.mult)
                tdot = sbs.tile([P, kmax], f32, tag="tdot")
                nc.vector.tensor_reduce(
                    out=tdot[:, 0:K],
                    in_=prod[:, 0:K * C2].rearrange("p (k c) -> p k c", k=K),
                    axis=X, op=Alu.add)
                u2 = sbs.tile([P, kmax], f32, tag="u2")
                nc.scalar.activation(out=u2[:, 0:K], in_=tdot[:, 0:K],
                                     func=Act.Exp)
                v2 = sbs.tile([P, kmax], f32, tag="v2")
                nc.scalar.activation(out=v2[:, 0:K], in_=tdot[:, 0:K],
                                     func=Act.Exp, scale=NEG_SLOPE)
                uf = sbs.tile([P, kmax], f32, tag="uf")
                nc.vector.tensor_scalar_mul(
                    out=uf[:, 0:K], in0=u2[:, 0:K],
                    scalar1=uvfg2[:, 3 * s + 1:3 * s + 2])
                selfw = uvfg2[:, 3 * s:3 * s + 1]
                wt = sbs.tile([P, kmax], f32, tag="wt")
                nc.vector.scalar_tensor_tensor(
                    out=wt[:, 0:K], in0=v2[:, 0:K],
                    scalar=uvfg2[:, 3 * s + 2:3 * s + 3],
                    in1=uf[:, 0:K], op0=Alu.mult, op1=Alu.max)
                den0 = sbs.tile([P, 1], f32, tag="den0")
                nc.vector.tensor_reduce(out=den0[:], in_=wt[:, 0:K],
                                        axis=X, op=Alu.add)
                den = sbs.tile([P, 1], f32, tag="den")
                nc.vector.tensor_tensor(out=den[:], in0=den0[:], in1=selfw,
                                        op=Alu.add)
                msgs = sbm.tile([P, kmax * C2], f32, tag="msgs")
                nc.vector.tensor_tensor(
                    out=msgs[:, 0:K * C2].rearrange("p (k c) -> p k c", c=C2),
                    in0=gv[:, :, 0:C2],
                    in1=wt[:, 0:K].rearrange("p k -> p k ()").to_broadcast(
                        [P, K, C2]),
                    op=Alu.mult)
                numr = sbs.tile([P, C2], f32, tag="numr")
                nc.vector.tensor_reduce(
                    out=numr[:],
                    in_=msgs[:, 0:K * C2].rearrange("p (k c) -> p c k", c=C2),
                    axis=X, op=Alu.add)
                rec = sbs.tile([P, 1], f32, tag="rec")
                nc.vector.reciprocal(out=rec[:], in_=den[:])
                num3 = sbs.tile([P, C2], f32, tag="num3")
                nc.vector.scalar_tensor_tensor(
                    out=num3[:], in0=ow[:, 0:C2], scalar=selfw,
                    in1=numr[:], op0=Alu.mult, op1=Alu.add)
                ob = sbs.tile([P, C2], f32, tag="ob")
                nc.vector.scalar_tensor_tensor(
                    out=ob[:], in0=num3[:], scalar=rec[:, 0:1],
                    in1=b2_sb[:], op0=Alu.mult, op1=Alu.add)
                nc.sync.dma_start(out=out2[s * P:(s + 1) * P, :], in_=ob[:])

    _split_overloaded_waits(nc)
    lower_extended_insts(nc)
    return nc


def _split_overloaded_waits(nc):
    """This walrus build accepts one sem wait per instruction; hoist extras
    onto NoOps spliced immediately before (same engine => same ordering)."""
    from concourse import mybir
    n_fix = 0
    for bb in nc.main_func.blocks:
        insts = bb.instructions
        out = []
        for ins in insts:
            si = getattr(ins, "sync_info", None)
            waits = list(si.on_wait) if (si and si.on_wait) else []
            if len(waits) > 1:
                si.on_wait = waits[-1:]
                rest = waits[:-1]
                while rest:
                    nop = mybir.InstNoOp(name=f"wsplit-{nc.next_id()}", ins=[], outs=[])
                    nop.engine = ins.engine
                    nop.sync_info = mybir.SyncInfo(on_wait=rest[:1], on_update=[])
                    rest = rest[1:]
                    out.append(nop)
                n_fix += 1
            out.append(ins)
        if len(out) != len(insts):
            insts.clear()
            insts.extend(out)
    return n_fix


# ---------------- entry point ----------------------------------------------
_LAST_EXEC_NS = None
_LAST_SCOPES = None


def kernel(x, edge_index, W1, att_src1, att_dst1, b1, W2, att_src2, att_dst2,
           b2, _trace=False):
    global _LAST_EXEC_NS, _LAST_SCOPES
    W1m, W2m = W1, W2
    _ensure_axon_hooks()
    import concourse.bass_utils as bass_utils
    bass_utils.upload_artifacts = lambda tmpdir: tmpdir
    from concourse.bass_utils import run_bass_kernel_spmd

    x = np.asarray(x, np.float32)
    src = np.asarray(edge_index[0], np.int64)
    dst = np.asarray(edge_index[1], np.int64)

    perm = _color_and_permute(src, dst)
    K_sched, idx16, soff = _build_slot_tables(src, dst, perm)

    w1c = _att_cat(np.asarray(W1m, np.float32),
                   np.asarray(att_src1, np.float32),
                   np.asarray(att_dst1, np.float32))
    w2c = _att_cat(np.asarray(W2m, np.float32),
                   np.asarray(att_src2, np.float32),
                   np.asarray(att_dst2, np.float32))
    b1r = np.asarray(b1, np.float32).reshape(1, HC1)
    b2r = np.asarray(b2, np.float32).reshape(1, C2)

    inv = np.empty(NPAD, np.int64)   # pid -> node
    inv[perm] = np.arange(NPAD)
    xp = np.zeros((NPAD, F_IN), np.float32)
    real = inv < N
    xp[real] = x[inv[real]]
    ident = np.eye(P, dtype=np.float32)
    # pad rows: h2 dot att_src2 -> -inf so exp()=0 (pad slots add nothing)
    as2vec = np.asarray(att_src2, np.float32).reshape(C2)
    padrow2 = (-1e18 * np.sign(as2vec)).astype(np.float32)
    padg2 = np.tile(padrow2, (2, 4)).astype(np.float32)

    nc = _build_program(K_sched, soff)
    in_maps = []
    for d in range(NCORES):
        xTs = np.ascontiguousarray(xp[d * NS:(d + 1) * NS].T)
        in_maps.append(dict(
            xTs=xTs, w1cat=w1c, w2cat=w2c, b1row=b1r, b2row=b2r,
            as2row=as2vec.reshape(1, C2),
            ident=ident, idx16=np.ascontiguousarray(idx16[d]),
            padg1=np.zeros((2, GW1), np.float32),
            padg2=padg2,
        ))
    res = run_bass_kernel_spmd(nc, in_maps, list(range(NCORES)), trace=_trace)
    _LAST_EXEC_NS = res.exec_time_ns
    _LAST_SCOPES = res.per_core_scope_times
    outp = np.concatenate([res.results[d]["out2"] for d in range(NCORES)], 0)
    out = np.empty((N, C2), np.float32)
    out[:] = outp[perm[:N]]
    return out


# revision 15
# speedup vs baseline: 1.8303x; 1.0226x over previous
"""Two-layer GAT (GATConv x2, PyG-style self-loops) on 8 Trainium2 cores.

Strategy (dst-major slots, batched dma_gather):
  - Nodes are permuted host-side: greedy 4-coloring balances each dst's
    in-edge sources across classes (class = pid % 4, needed because
    dma_gather indices are int16: idx = src_pid // 4 <= 25088); nodes are
    then sorted by class-count vector so 128-node dst blocks are
    degree- and class-homogeneous, which minimizes slot padding.
  - Per dst slot: edge slots [128 dst x K], gathered with one dma_gather
    call per class (<=8 k-columns per call), thousands of descriptors per
    call instead of one SWDGE launch per 128 edges.
  - Node table rows: layer-1 [h(128) | u(2) v(2) f(2) g(2)] = 136 floats
    (gather elem 192 floats, %256B), layer-2 [h2(64) | u2 v2] = 66 floats
    (elem 128), where u=e^{a_s}, v=e^{0.2 a_s}, f=e^{a_d}, g=e^{0.2 a_d}:
    exp(leaky_relu(a_s+a_d)) == max(u*f, v*g) exactly, so per-edge softmax
    weights are broadcast mults and a max; storing u2/v2 removes the
    per-edge layer-2 attention dot entirely.
  - Per slot, both heads fused per DVE op: uf/vg [P,K,2] mults, w = max,
    one strided reduce for den, one 3D-broadcast mult for w*h, one strided
    reduce for the numerator. Self-loop and bias fold into
    scalar_tensor_tensor epilogues; copies/exps run on the Scalar engine.
  - p1 computes the core's own shard table (98 matmuls), AllGathers it;
    the layer-2 node transform (ELU -> transpose -> @W2cat -> row) is fused
    into e1's epilogue, then a second AllGather shares the layer-2 table.

kernel() takes full inputs, returns the full [100000, 64] output (fp32
everywhere: the rel-err metric's 1e-3 floor leaves ~2e-5 abs budget).
"""
import sys
from contextlib import ExitStack

import numpy as np

# ---------------- problem constants (hardcoded per harness contract) -------
N = 100000
NCORES = 8
P = 128
F_IN = 128
H1 = 2
C1 = 64
HC1 = 128
C2 = 64
NS = 12544          # nodes per core shard = 98 * 128
NSLOT = NS // P     # 98 dst slots per core
NPAD = NS * NCORES  # 100352
NBLK = NPAD // P    # 784
W1R = 144           # layer-1 row: h(128) u(2) v(2) f(2) g(2) pad(8)
GW1 = 4 * W1R       # 576 floats per 4-row group (%64 floats)
EW1 = 192           # gather elem floats (>=136, %64)
W2R = 64            # layer-2 row: h2 only (u2/v2 computed on-chip)
GW2 = 4 * W2R       # 256 (%64)
EW2 = 64            # %64
GRP = NS // 4 + 2   # groups per core shard incl. 2 zero pad groups
NG = NCORES * GRP   # full table groups (AllGather of 8 shards)
NEG_SLOPE = 0.2


def _ensure_axon_hooks():
    """bass_utils' trace path needs antenv.axon_hooks; provide it if absent."""
    import types
    try:
        import antenv.axon_hooks as mod
    except ImportError:
        import antenv
        mod = types.ModuleType("antenv.axon_hooks")
        mod._hook = None
        def set_axon_ntff_profile_hook(hook):
            mod._hook = hook
        def get_axon_ntff_profile_hook():
            return mod._hook
        mod.set_axon_ntff_profile_hook = set_axon_ntff_profile_hook
        mod.get_axon_ntff_profile_hook = get_axon_ntff_profile_hook
        sys.modules["antenv.axon_hooks"] = mod
        antenv.axon_hooks = mod
    if mod.get_axon_ntff_profile_hook() is None:
        try:
            from trn_agent_boot.trn_boot import _ntff_profile_via_ctypes
            hook = _ntff_profile_via_ctypes("/opt/axon/libaxon_pjrt.so")
            if hook is not None:
                mod.set_axon_ntff_profile_hook(hook)
        except Exception:
            pass


# ---------------- host-side graph preprocessing ----------------------------
def _color_and_permute(src, dst):
    """Greedy 4-coloring (balance each dst's sources across classes) and a
    node permutation: pid % 4 == class, blocks sorted by class-count vector.
    Returns perm (node -> pid)."""
    odeg = np.bincount(src, minlength=NPAD)
    proc = np.argsort(-odeg, kind="stable")
    es = np.argsort(src, kind="stable")
    ss, dd = src[es], dst[es]
    starts = np.searchsorted(ss, np.arange(NPAD + 1))
    cnt = np.zeros((NPAD, 4), np.int32)     # per-dst class counts
    quota = np.full(4, NPAD // 4, np.int64)
    color = np.full(NPAD, -1, np.int8)
    has_out = odeg > 0
    for n in proc:
        if not has_out[n]:
            break  # proc is sorted by out-degree desc
        a, b = starts[n], starts[n + 1]
        nb = dd[a:b]
        score = cnt[nb].sum(0).astype(np.float64)
        score += (1.0 - quota / (NPAD // 4)) * 0.5
        score[quota <= 0] = np.inf
        c = int(np.argmin(score))
        color[n] = c
        quota[c] -= 1
        cnt[nb, c] += 1
    # refinement: move a node's color where it most reduces its dsts'
    # class-count imbalance
    for _ in range(5):
        for n in np.flatnonzero(has_out):
            a, b = starts[n], starts[n + 1]
            nb = dd[a:b]
            c1 = color[n]
            s = cnt[nb].sum(0)
            c2 = int(np.argmin(s))
            if c2 != c1 and s[c2] + len(nb) < s[c1]:
                cnt[nb, c1] -= 1
                cnt[nb, c2] += 1
                color[n] = c2
    sizes = np.bincount(color[color >= 0], minlength=4)
    while sizes.max() > NPAD // 4:
        c1 = int(sizes.argmax())
        cand = np.flatnonzero(color == c1)
        n = cand[np.argmin(odeg[cand])]
        c2 = int(sizes.argmin())
        a, b = starts[n], starts[n + 1]
        cnt[dd[a:b], c1] -= 1
        cnt[dd[a:b], c2] += 1
        color[n] = c2
        sizes[c1] -= 1
        sizes[c2] += 1
    left = np.flatnonzero(color < 0)
    fill = np.repeat(np.arange(4), (NPAD // 4 - sizes).clip(0))[:len(left)]
    color[left] = fill
    assert (np.bincount(color, minlength=4) == NPAD // 4).all()
    mx = cnt.max(1)
    am = cnt.argmax(1)
    key = np.lexsort((cnt[:, 3], cnt[:, 2], cnt[:, 1], cnt[:, 0], am, mx))
    # sorted block rank r -> (core r%8, slot r//8); within a block position
    # p takes color p%4 (pid%4 == p%4).
    queues = [key[color[key] == c] for c in range(4)]
    pos = [0, 0, 0, 0]
    perm = np.empty(NPAD, np.int64)   # node -> pid
    for g in range(NPAD):
        r, p = g // P, g % P
        c = p % 4
        n = queues[c][pos[c]]
        pos[c] += 1
        perm[n] = (r % NCORES) * NS + (r // NCORES) * P + p
    return perm


def _build_slot_tables(src, dst, perm):
    """Per-core, per-slot, per-class edge slot tables in pid space.
    K_sched[s][c] = max over the 8 cores of the block max per-partition
    class count. Returns K_sched [98][4] and per-core int16 idx streams."""
    psrc = perm[src]
    pdst = perm[dst]
    blk = pdst // P
    part = pdst % P
    cls = psrc % 4
    cnt = np.zeros((NPAD, 4), np.int32)
    np.add.at(cnt, (pdst, cls), 1)
    bmax = cnt.reshape(NBLK, P, 4).max(1)             # [784, 4]
    # pid block b: core b % 8 (rank r=b? no: pid = (r%8)*NS + (r//8)*128+p)
    # => block index in pid space: b = core*NSLOT + slot
    K_sched = bmax.reshape(NBLK // NSLOT, NSLOT, 4).max(0)  # wrong axis fix below
    # recompute correctly: pid block b: core b // NSLOT, slot b % NSLOT
    K_sched = bmax.reshape(NCORES, NSLOT, 4).max(0)   # [98, 4]
    ktot = K_sched.sum(1)

    order = np.lexsort((part, cls, blk))
    pb, pc, pp, ps = blk[order], cls[order], part[order], psrc[order]
    key = (pb * 4 + pc) * P + pp
    counts = np.bincount(key, minlength=NBLK * 4 * P)
    kpos = np.arange(len(order)) - np.repeat(
        np.concatenate([[0], np.cumsum(counts)[:-1]]), counts)

    # idx value = table group row: shards AllGathered with 2 zero pad groups
    # per core appended, so group g lands at row g + 2*(g//(NS//4));
    # pad slots point at core 0's zero rows (row NS//4).
    PADG = NS // 4
    koff = np.zeros((NSLOT, 4), np.int64)
    koff[:, 1:] = np.cumsum(K_sched, 1)[:, :-1]
    soff = np.concatenate([[0], np.cumsum(ktot)])
    total_cols = int(soff[-1])
    idx = np.full((NCORES, total_cols * P), PADG, np.int32)
    core = pb // NSLOT
    slot = pb % NSLOT
    pos_in_stream = (soff[slot] + koff[slot, pc] + kpos) * P + pp
    grp = ps // 4
    idx[core, pos_in_stream] = grp + 2 * (grp // (NS // 4))
    # wrap in 16 partitions: position i -> [i%16, i//16]; replicate 8x.
    idx16 = np.ascontiguousarray(
        idx.reshape(NCORES, total_cols * P // 16, 16).transpose(0, 2, 1)
    ).astype(np.int16)
    idx16 = np.ascontiguousarray(np.tile(idx16, (1, 8, 1)))
    return K_sched, idx16, soff


def _att_cat(W, att_src, att_dst):
    h, c = att_src.shape
    cin = W.shape[1]
    As = np.zeros((cin, h), np.float32)
    Ad = np.zeros((cin, h), np.float32)
    for i in range(h):
        As[i * c:(i + 1) * c, i] = att_src[i]
        Ad[i * c:(i + 1) * c, i] = att_dst[i]
    return np.concatenate([W, W @ As, W @ Ad], 1).astype(np.float32)


# ---------------- bass program --------------------------------------------
def _build_program(K_sched, soff):
    import concourse.bass as bass
    import concourse.tile as tile
    from concourse import mybir, library_config
    from concourse.library_overlay import lower_extended_insts
    from concourse.vector_clock import ScopedClock

    f32 = mybir.dt.float32
    i16 = mybir.dt.int16
    Act = mybir.ActivationFunctionType
    Alu = mybir.AluOpType
    X = mybir.AxisListType.X

    total_cols = int(soff[-1])
    kmax = int(max(K_sched.sum(1)))

    class PatchedTileContext(tile.TileContext):
        """Kernel-tail drain must not carry more waits than the ISA allows;
        split them across chained drains (this walrus allows 1 wait/inst)."""
        def _drain_and_barrier(self, tick_clock, wait_clock):
            drain_inst = self.nc.sync.drain()
            wait_clock.add_sem_waits(
                drain_inst.ins, ScopedClock({None: tick_clock.global_clock})
            )
            si = drain_inst.ins.sync_info
            if si is not None and si.on_wait and len(si.on_wait) > 1:
                waits = list(si.on_wait)
                si.on_wait = waits[:1]
                rest = waits[1:]
                while rest:
                    extra = self.nc.sync.drain()
                    extra.ins.sync_info = mybir.SyncInfo(on_wait=rest[:1], on_update=[])
                    rest = rest[1:]
            self.nc.all_engine_barrier()
            assert self.sems is not None
            popped = self.nc._tile_sem_poison_stack.pop()
            assert popped is self._sem_poison
            self.nc.clear_and_free_semaphores(list(self.sems.allocated().values()))
            self.nc.all_engine_barrier()

    nc = bass.Bass(num_devices=NCORES, num_swdge_queues=4)

    xTs = nc.declare_dram_parameter("xTs", [P, NS], f32, isOutput=False)
    w1cat = nc.declare_dram_parameter("w1cat", [P, HC1 + 2 * H1], f32, isOutput=False)
    w2cat = nc.declare_dram_parameter("w2cat", [P, C2 + 2], f32, isOutput=False)
    b1row = nc.declare_dram_parameter("b1row", [1, HC1], f32, isOutput=False)
    b2row = nc.declare_dram_parameter("b2row", [1, C2], f32, isOutput=False)
    as2row = nc.declare_dram_parameter("as2row", [1, C2], f32, isOutput=False)
    ident_in = nc.declare_dram_parameter("ident", [P, P], f32, isOutput=False)
    idx_in = nc.declare_dram_parameter("idx16", [P, total_cols * P // 16],
                                       i16, isOutput=False)
    padg1 = nc.declare_dram_parameter("padg1", [2, GW1], f32, isOutput=False)
    padg2 = nc.declare_dram_parameter("padg2", [2, GW2], f32, isOutput=False)
    out2 = nc.declare_dram_parameter("out2", [NS, C2], f32, isOutput=True)

    with PatchedTileContext(nc) as tc, ExitStack() as ctx:
        nc.gpsimd.load_library(library_config.mlp)
        const = ctx.enter_context(tc.tile_pool(name="const", bufs=1))
        dram = ctx.enter_context(tc.tile_pool(name="dram", bufs=1, space="DRAM"))

        tab1s = dram.tile([GRP, GW1], f32)                     # own shard L1
        tab1 = dram.tile([NG, GW1], f32, addr_space="Shared")  # full L1
        tab2s = dram.tile([GRP, GW2], f32)
        tab2 = dram.tile([NG, GW2], f32, addr_space="Shared")

        w1_sb = const.tile([P, HC1 + 2 * H1], f32)
        nc.sync.dma_start(out=w1_sb[:], in_=w1cat[:])
        w2_sb = const.tile([P, C2 + 2], f32)
        nc.sync.dma_start(out=w2_sb[:], in_=w2cat[:])
        b1_sb = const.tile([P, HC1], f32)
        nc.sync.dma_start(out=b1_sb[:], in_=b1row[0:1, :].to_broadcast([P, HC1]))
        b2_sb = const.tile([P, C2], f32)
        nc.sync.dma_start(out=b2_sb[:], in_=b2row[0:1, :].to_broadcast([P, C2]))
        as2_sb = const.tile([P, C2], f32)
        nc.sync.dma_start(out=as2_sb[:], in_=as2row[0:1, :].to_broadcast([P, C2]))
        ident_sb = const.tile([P, P], f32)
        nc.sync.dma_start(out=ident_sb[:], in_=ident_in[:])
        idx_sb = const.tile([P, total_cols * P // 16], i16)
        nc.sync.dma_start(out=idx_sb[:], in_=idx_in[:])
        uvfg2 = const.tile([P, NSLOT * 3], f32)   # per-slot [selfw2, f2, g2]
        sw1 = const.tile([P, NSLOT * 2], f32)     # per-node layer-1 self weight
        # zero pad groups at each shard's tail (u=v=0 -> pad slots add 0)
        nc.sync.dma_start(out=tab1s[GRP - 2:GRP, :], in_=padg1[:])
        nc.sync.dma_start(out=tab2s[GRP - 2:GRP, :], in_=padg2[:])

        # ---- p1: own-shard node transform -> tab1s ----
        with nc.named_scope("p1"), ExitStack() as c2:
            sbp = c2.enter_context(tc.tile_pool(name="p1sb", bufs=3))
            psp = c2.enter_context(tc.tile_pool(name="p1ps", bufs=3, space="PSUM"))
            for sl in range(14):
                slab = sbp.tile([P, 7 * P], f32, tag="slab")
                nc.sync.dma_start(out=slab[:], in_=xTs[:, sl * 896:(sl + 1) * 896])
                for k in range(7):
                    s = sl * 7 + k
                    ps = psp.tile([P, HC1 + 2 * H1], f32, tag="ps")
                    nc.tensor.matmul(out=ps[:], lhsT=slab[:, k * P:(k + 1) * P],
                                     rhs=w1_sb[:], start=True, stop=True)
                    t1 = sbp.tile([P, W1R], f32, tag="t1")
                    nc.scalar.copy(out=t1[:, 0:HC1], in_=ps[:, 0:HC1])
                    nc.scalar.activation(out=t1[:, 128:130],
                                         in_=ps[:, 128:130], func=Act.Exp)
                    nc.scalar.activation(out=t1[:, 130:132],
                                         in_=ps[:, 128:130], func=Act.Exp,
                                         scale=NEG_SLOPE)
                    nc.scalar.activation(out=t1[:, 132:134],
                                         in_=ps[:, 130:132], func=Act.Exp)
                    nc.scalar.activation(out=t1[:, 134:136],
                                         in_=ps[:, 130:132], func=Act.Exp,
                                         scale=NEG_SLOPE)
                    nc.vector.memset(t1[:, 136:W1R], 0.0)
                    sa1 = sbp.tile([P, 4], f32, tag="sa1")
                    nc.vector.tensor_tensor(out=sa1[:, 0:2], in0=t1[:, 128:130],
                                            in1=t1[:, 132:134], op=Alu.mult)
                    nc.vector.tensor_tensor(out=sa1[:, 2:4], in0=t1[:, 130:132],
                                            in1=t1[:, 134:136], op=Alu.mult)
                    nc.vector.tensor_tensor(out=sw1[:, 2 * s:2 * s + 2],
                                            in0=sa1[:, 0:2], in1=sa1[:, 2:4],
                                            op=Alu.max)
                    nc.sync.dma_start(
                        out=tab1s[s * 32:(s + 1) * 32, :].rearrange(
                            "g (r w) -> (g r) w", r=4),
                        in_=t1[:])

        # ---- AllGather layer-1 table ----
        nc.gpsimd.collective_compute(
            "AllGather", mybir.AluOpType.bypass,
            replica_groups=[list(range(NCORES))],
            ins=[tab1s[:, :].opt()],
            outs=[tab1[:, :].opt()],
        )

        reg_cache = {}
        qctr = [0]

        def nreg(n):
            if n not in reg_cache:
                reg_cache[n] = nc.gpsimd.to_reg(n)
            return reg_cache[n]

        def issue_gather(pool, s, tab, gw, ew, w_row):
            ktot = int(K_sched[s].sum())
            g = pool.tile([P, kmax * ew], f32, tag="g")
            co = 0
            for c in range(4):
                kc = int(K_sched[s][c])
                k0 = 0
                while k0 < kc:
                    kch = min(kc - k0, 8)   # <=1024 descriptors per call
                    n_idx = P * kch
                    ioff = (int(soff[s]) + co) * P // 16
                    in_ap = bass.AP(tab[:, :].tensor, c * w_row,
                                    [[gw, NG - 1], [1, ew]])
                    nc.gpsimd.dma_gather(
                        out_ap=g[:, co * ew:(co + kch) * ew].rearrange(
                            "p (k e) -> p k e", k=kch),
                        in_ap=in_ap,
                        idxs_ap=idx_sb[:, ioff:ioff + n_idx // 16],
                        num_idxs=n_idx, num_idxs_reg=nreg(n_idx),
                        elem_size=ew, elem_step=gw,
                        queue_num=qctr[0] % 4,
                    )
                    qctr[0] += 1
                    co += kch
                    k0 += kch
            return g, ktot

        # ---- e1: layer-1 edge phase (emits layer-2 table rows) ----
        with nc.named_scope("e1"), ExitStack() as c2:
            sbg = c2.enter_context(tc.tile_pool(name="e1g", bufs=3))
            sbo = c2.enter_context(tc.tile_pool(name="e1o", bufs=4))
            sbm = c2.enter_context(tc.tile_pool(name="e1m", bufs=2))
            sbs = c2.enter_context(tc.tile_pool(name="e1s", bufs=3))
            psp = c2.enter_context(tc.tile_pool(name="e1ps", bufs=2, space="PSUM"))

            def issue1(s):
                g, ktot = issue_gather(sbg, s, tab1, GW1, EW1, W1R)
                ow = sbo.tile([P, W1R], f32, tag="ow")
                nc.sync.dma_start(
                    out=ow[:],
                    in_=tab1s[s * 32:(s + 1) * 32, :].rearrange(
                        "g (r w) -> (g r) w", r=4))
                return g, ktot, ow

            PF = 3
            pre = [issue1(s) for s in range(PF)]
            for s in range(NSLOT):
                g, K, ow = pre.pop(0)
                if s + PF < NSLOT:
                    pre.append(issue1(s + PF))
                gv = g[:, 0:K * EW1].rearrange("p (k w) -> p k w", w=EW1)
                # per-edge uf/vg for both heads: [P, K, 2]
                uf2 = sbs.tile([P, 2 * kmax], f32, tag="uf2")
                nc.vector.tensor_tensor(
                    out=uf2[:, 0:2 * K].rearrange("p (k h) -> p k h", h=2),
                    in0=gv[:, :, 128:130],
                    in1=ow[:, 132:134].rearrange("p h -> p () h").to_broadcast(
                        [P, K, 2]),
                    op=Alu.mult)
                vg2 = sbs.tile([P, 2 * kmax], f32, tag="vg2")
                nc.vector.tensor_tensor(
                    out=vg2[:, 0:2 * K].rearrange("p (k h) -> p k h", h=2),
                    in0=gv[:, :, 130:132],
                    in1=ow[:, 134:136].rearrange("p h -> p () h").to_broadcast(
                        [P, K, 2]),
                    op=Alu.mult)
                selfw = sw1[:, 2 * s:2 * s + 2]
                # w = max(uf, vg); den = self_w + sum_k w
                wt2 = sbs.tile([P, 2 * kmax], f32, tag="wt2")
                nc.vector.tensor_tensor(out=wt2[:, 0:2 * K], in0=uf2[:, 0:2 * K],
                                        in1=vg2[:, 0:2 * K], op=Alu.max)
                den0 = sbs.tile([P, 2], f32, tag="den0")
                nc.vector.tensor_reduce(
                    out=den0[:],
                    in_=wt2[:, 0:2 * K].rearrange("p (k h) -> p h k", h=2),
                    axis=X, op=Alu.add)
                den = sbs.tile([P, 2], f32, tag="den")
                nc.vector.tensor_tensor(out=den[:], in0=den0[:], in1=selfw,
                                        op=Alu.add)
                # messages and numerator (both heads in one op each)
                msgs = sbm.tile([P, kmax * HC1], f32, tag="msgs")
                nc.vector.tensor_tensor(
                    out=msgs[:, 0:K * HC1].rearrange(
                        "p (k h c) -> p k h c", h=2, c=C1),
                    in0=gv[:, :, 0:HC1].rearrange(
                        "p k (h c) -> p k h c", c=C1),
                    in1=wt2[:, 0:2 * K].rearrange(
                        "p (k h) -> p k h ()", h=2).to_broadcast([P, K, 2, C1]),
                    op=Alu.mult)
                numr = sbs.tile([P, HC1], f32, tag="numr")
                nc.vector.tensor_reduce(
                    out=numr[:],
                    in_=msgs[:, 0:K * HC1].rearrange("p (k c) -> p c k", c=HC1),
                    axis=X, op=Alu.add)
                rec = sbs.tile([P, 2], f32, tag="rec")
                nc.vector.reciprocal(out=rec[:], in_=den[:])
                ob = sbs.tile([P, HC1], f32, tag="ob")
                for h in range(2):
                    cs = slice(h * C1, (h + 1) * C1)
                    num3 = sbs.tile([P, C1], f32, tag=f"num3{h}")
                    nc.vector.scalar_tensor_tensor(
                        out=num3[:], in0=ow[:, cs], scalar=sw1[:, 2 * s + h:2 * s + h + 1],
                        in1=numr[:, cs], op0=Alu.mult, op1=Alu.add)
                    nc.vector.scalar_tensor_tensor(
                        out=ob[:, cs], in0=num3[:], scalar=rec[:, h:h + 1],
                        in1=b1_sb[:, cs], op0=Alu.mult, op1=Alu.add)
                # ELU -> transpose -> @W2cat -> layer-2 table row
                negm = sbs.tile([P, HC1], f32, tag="negm")
                nc.vector.tensor_scalar_min(out=negm[:], in0=ob[:], scalar1=0.0)
                pos = sbs.tile([P, HC1], f32, tag="pos")
                nc.scalar.activation(out=pos[:], in_=ob[:], func=Act.Relu)
                em = sbs.tile([P, HC1], f32, tag="em")
                nc.scalar.activation(out=em[:], in_=negm[:], func=Act.Exp)
                h1b = sbs.tile([P, HC1], f32, tag="h1b")
                nc.vector.scalar_tensor_tensor(
                    out=h1b[:], in0=em[:], scalar=-1.0, in1=pos[:],
                    op0=Alu.add, op1=Alu.add)
                ps_t = psp.tile([P, P], f32, tag="pst")
                nc.tensor.transpose(out=ps_t[:], in_=h1b[:], identity=ident_sb[:])
                h1t = sbs.tile([P, P], f32, tag="h1t")
                nc.scalar.copy(out=h1t[:], in_=ps_t[:])
                ps2 = psp.tile([P, C2 + 2], f32, tag="ps2")
                nc.tensor.matmul(out=ps2[:], lhsT=h1t[:], rhs=w2_sb[:],
                                 start=True, stop=True)
                t2 = sbs.tile([P, W2R], f32, tag="t2")
                nc.scalar.copy(out=t2[:, 0:C2], in_=ps2[:, 0:C2])
                nc.scalar.activation(out=uvfg2[:, 3 * s + 1:3 * s + 2],
                                     in_=ps2[:, C2 + 1:C2 + 2], func=Act.Exp)
                nc.scalar.activation(out=uvfg2[:, 3 * s + 2:3 * s + 3],
                                     in_=ps2[:, C2 + 1:C2 + 2], func=Act.Exp,
                                     scale=NEG_SLOPE)
                se2 = sbs.tile([P, 4], f32, tag="se2")
                nc.scalar.activation(out=se2[:, 0:1], in_=ps2[:, C2:C2 + 1],
                                     func=Act.Exp)
                nc.scalar.activation(out=se2[:, 1:2], in_=ps2[:, C2:C2 + 1],
                                     func=Act.Exp, scale=NEG_SLOPE)
                sm2 = sbs.tile([P, 2], f32, tag="sm2")
                nc.vector.tensor_tensor(out=sm2[:, 0:1], in0=se2[:, 0:1],
                                        in1=uvfg2[:, 3 * s + 1:3 * s + 2],
                                        op=Alu.mult)
                nc.vector.tensor_tensor(out=sm2[:, 1:2], in0=se2[:, 1:2],
                                        in1=uvfg2[:, 3 * s + 2:3 * s + 3],
                                        op=Alu.mult)
                nc.vector.tensor_tensor(out=uvfg2[:, 3 * s:3 * s + 1],
                                        in0=sm2[:, 0:1], in1=sm2[:, 1:2],
                                        op=Alu.max)
                nc.sync.dma_start(
                    out=tab2s[s * 32:(s + 1) * 32, :].rearrange(
                        "g (r w) -> (g r) w", r=4),
                    in_=t2[:])

        # ---- AllGather layer-2 table ----
        nc.gpsimd.collective_compute(
            "AllGather", mybir.AluOpType.bypass,
            replica_groups=[list(range(NCORES))],
            ins=[tab2s[:, :].opt()],
            outs=[tab2[:, :].opt()],
        )

        # ---- e2: layer-2 edge phase -> out2 ----
        with nc.named_scope("e2"), ExitStack() as c2:
            sbg = c2.enter_context(tc.tile_pool(name="e2g", bufs=7))
            sbo = c2.enter_context(tc.tile_pool(name="e2o", bufs=8))
            sbm = c2.enter_context(tc.tile_pool(name="e2m", bufs=2))
            sbs = c2.enter_context(tc.tile_pool(name="e2s", bufs=3))

            def issue2(s):
                g, ktot = issue_gather(sbg, s, tab2, GW2, EW2, W2R)
                ow = sbo.tile([P, W2R], f32, tag="ow")
                nc.sync.dma_start(
                    out=ow[:],
                    in_=tab2s[s * 32:(s + 1) * 32, :].rearrange(
                        "g (r w) -> (g r) w", r=4))
                return g, ktot, ow

            PF = 6
            pre = [issue2(s) for s in range(PF)]
            for s in range(NSLOT):
                g, K, ow = pre.pop(0)
                if s + PF < NSLOT:
                    pre.append(issue2(s + PF))
                gv = g[:, 0:K * EW2].rearrange("p (k w) -> p k w", w=EW2)
                # per-edge a_s2 dot, then u2/v2 via exp
                prod = sbm.tile([P, kmax * C2], f32, tag="prod")
                nc.vector.tensor_tensor(
                    out=prod[:, 0:K * C2].rearrange("p (k c) -> p k c", c=C2),
                    in0=gv[:, :, 0:C2],
                    in1=as2_sb[:].rearrange("p c -> p () c").to_broadcast(
                        [P, K, C2]),
                    op=Alu.mult)
                tdot = sbs.tile([P, kmax], f32, tag="tdot")
                nc.vector.tensor_reduce(
                    out=tdot[:, 0:K],
                    in_=prod[:, 0:K * C2].rearrange("p (k c) -> p k c", k=K),
                    axis=X, op=Alu.add)
                u2 = sbs.tile([P, kmax], f32, tag="u2")
                nc.scalar.activation(out=u2[:, 0:K], in_=tdot[:, 0:K],
                                     func=Act.Exp)
                v2 = sbs.tile([P, kmax], f32, tag="v2")
                nc.scalar.activation(out=v2[:, 0:K], in_=tdot[:, 0:K],
                                     func=Act.Exp, scale=NEG_SLOPE)
                uf = sbs.tile([P, kmax], f32, tag="uf")
                nc.vector.tensor_scalar_mul(
                    out=uf[:, 0:K], in0=u2[:, 0:K],
                    scalar1=uvfg2[:, 3 * s + 1:3 * s + 2])
                selfw = uvfg2[:, 3 * s:3 * s + 1]
                wt = sbs.tile([P, kmax], f32, tag="wt")
                nc.vector.scalar_tensor_tensor(
                    out=wt[:, 0:K], in0=v2[:, 0:K],
                    scalar=uvfg2[:, 3 * s + 2:3 * s + 3],
                    in1=uf[:, 0:K], op0=Alu.mult, op1=Alu.max)
                den0 = sbs.tile([P, 1], f32, tag="den0")
                nc.vector.tensor_reduce(out=den0[:], in_=wt[:, 0:K],
                                        axis=X, op=Alu.add)
                den = sbs.tile([P, 1], f32, tag="den")
                nc.vector.tensor_tensor(out=den[:], in0=den0[:], in1=selfw,
                                        op=Alu.add)
                msgs = sbm.tile([P, kmax * C2], f32, tag="msgs")
                nc.vector.tensor_tensor(
                    out=msgs[:, 0:K * C2].rearrange("p (k c) -> p k c", c=C2),
                    in0=gv[:, :, 0:C2],
                    in1=wt[:, 0:K].rearrange("p k -> p k ()").to_broadcast(
                        [P, K, C2]),
                    op=Alu.mult)
                numr = sbs.tile([P, C2], f32, tag="numr")
                nc.vector.tensor_reduce(
                    out=numr[:],
                    in_=msgs[:, 0:K * C2].rearrange("p (k c) -> p c k", c=C2),
                    axis=X, op=Alu.add)
                rec = sbs.tile([P, 1], f32, tag="rec")
                nc.vector.reciprocal(out=rec[:], in_=den[:])
                num3 = sbs.tile([P, C2], f32, tag="num3")
                nc.vector.scalar_tensor_tensor(
                    out=num3[:], in0=ow[:, 0:C2], scalar=selfw,
                    in1=numr[:], op0=Alu.mult, op1=Alu.add)
                ob = sbs.tile([P, C2], f32, tag="ob")
                nc.vector.scalar_tensor_tensor(
                    out=ob[:], in0=num3[:], scalar=rec[:, 0:1],
                    in1=b2_sb[:], op0=Alu.mult, op1=Alu.add)
                nc.sync.dma_start(out=out2[s * P:(s + 1) * P, :], in_=ob[:])

    _split_overloaded_waits(nc)
    lower_extended_insts(nc)
    return nc


def _split_overloaded_waits(nc):
    """This walrus build accepts one sem wait per instruction; hoist extras
    onto NoOps spliced immediately before (same engine => same ordering)."""
    from concourse import mybir
    n_fix = 0
    for bb in nc.main_func.blocks:
        insts = bb.instructions
        out = []
        for ins in insts:
            si = getattr(ins, "sync_info", None)
            waits = list(si.on_wait) if (si and si.on_wait) else []
            if len(waits) > 1:
                si.on_wait = waits[-1:]
                rest = waits[:-1]
                while rest:
                    nop = mybir.InstNoOp(name=f"wsplit-{nc.next_id()}", ins=[], outs=[])
                    nop.engine = ins.engine
                    nop.sync_info = mybir.SyncInfo(on_wait=rest[:1], on_update=[])
                    rest = rest[1:]
                    out.append(nop)
                n_fix += 1
            out.append(ins)
        if len(out) != len(insts):
            insts.clear()
            insts.extend(out)
    return n_fix


# ---------------- entry point ----------------------------------------------
_LAST_EXEC_NS = None
_LAST_SCOPES = None


def kernel(x, edge_index, W1, att_src1, att_dst1, b1, W2, att_src2, att_dst2,
           b2, _trace=False):
    global _LAST_EXEC_NS, _LAST_SCOPES
    W1m, W2m = W1, W2
    _ensure_axon_hooks()
    import concourse.bass_utils as bass_utils
    bass_utils.upload_artifacts = lambda tmpdir: tmpdir
    from concourse.bass_utils import run_bass_kernel_spmd

    x = np.asarray(x, np.float32)
    src = np.asarray(edge_index[0], np.int64)
    dst = np.asarray(edge_index[1], np.int64)

    perm = _color_and_permute(src, dst)
    K_sched, idx16, soff = _build_slot_tables(src, dst, perm)

    w1c = _att_cat(np.asarray(W1m, np.float32),
                   np.asarray(att_src1, np.float32),
                   np.asarray(att_dst1, np.float32))
    w2c = _att_cat(np.asarray(W2m, np.float32),
                   np.asarray(att_src2, np.float32),
                   np.asarray(att_dst2, np.float32))
    b1r = np.asarray(b1, np.float32).reshape(1, HC1)
    b2r = np.asarray(b2, np.float32).reshape(1, C2)

    inv = np.empty(NPAD, np.int64)   # pid -> node
    inv[perm] = np.arange(NPAD)
    xp = np.zeros((NPAD, F_IN), np.float32)
    real = inv < N
    xp[real] = x[inv[real]]
    ident = np.eye(P, dtype=np.float32)
    # pad rows: h2 dot att_src2 -> -inf so exp()=0 (pad slots add nothing)
    as2vec = np.asarray(att_src2, np.float32).reshape(C2)
    padrow2 = (-1e18 * np.sign(as2vec)).astype(np.float32)
    padg2 = np.tile(padrow2, (2, 4)).astype(np.float32)

    nc = _build_program(K_sched, soff)
    in_maps = []
    for d in range(NCORES):
        xTs = np.ascontiguousarray(xp[d * NS:(d + 1) * NS].T)
        in_maps.append(dict(
            xTs=xTs, w1cat=w1c, w2cat=w2c, b1row=b1r, b2row=b2r,
            as2row=as2vec.reshape(1, C2),
            ident=ident, idx16=np.ascontiguousarray(idx16[d]),
            padg1=np.zeros((2, GW1), np.float32),
            padg2=padg2,
        ))
    res = run_bass_kernel_spmd(nc, in_maps, list(range(NCORES)), trace=_trace)
    _LAST_EXEC_NS = res.exec_time_ns
    _LAST_SCOPES = res.per_core_scope_times
    outp = np.concatenate([res.results[d]["out2"] for d in range(NCORES)], 0)
    out = np.empty((N, C2), np.float32)
    out[:] = outp[perm[:N]]
    return out


# revision 17
# speedup vs baseline: 1.8609x; 1.0167x over previous
"""Two-layer GAT (GATConv x2, PyG-style self-loops) on 8 Trainium2 cores.

Strategy (dst-major slots, batched dma_gather):
  - Nodes are permuted host-side: greedy 4-coloring balances each dst's
    in-edge sources across classes (class = pid % 4, needed because
    dma_gather indices are int16: idx = src_pid // 4 <= 25088); nodes are
    then sorted by class-count vector so 128-node dst blocks are
    degree- and class-homogeneous, which minimizes slot padding.
  - Per dst slot: edge slots [128 dst x K], gathered with one dma_gather
    call per class (<=8 k-columns per call), thousands of descriptors per
    call instead of one SWDGE launch per 128 edges.
  - Node table rows: layer-1 [h(128) | u(2) v(2) f(2) g(2)] = 136 floats
    (gather elem 192 floats, %256B), layer-2 [h2(64) | u2 v2] = 66 floats
    (elem 128), where u=e^{a_s}, v=e^{0.2 a_s}, f=e^{a_d}, g=e^{0.2 a_d}:
    exp(leaky_relu(a_s+a_d)) == max(u*f, v*g) exactly, so per-edge softmax
    weights are broadcast mults and a max; storing u2/v2 removes the
    per-edge layer-2 attention dot entirely.
  - Per slot, both heads fused per DVE op: uf/vg [P,K,2] mults, w = max,
    one strided reduce for den, one 3D-broadcast mult for w*h, one strided
    reduce for the numerator. Self-loop and bias fold into
    scalar_tensor_tensor epilogues; copies/exps run on the Scalar engine.
  - p1 computes the core's own shard table (98 matmuls), AllGathers it;
    the layer-2 node transform (ELU -> transpose -> @W2cat -> row) is fused
    into e1's epilogue, then a second AllGather shares the layer-2 table.

kernel() takes full inputs, returns the full [100000, 64] output (fp32
everywhere: the rel-err metric's 1e-3 floor leaves ~2e-5 abs budget).
"""
import sys
from contextlib import ExitStack

import numpy as np

# ---------------- problem constants (hardcoded per harness contract) -------
N = 100000
NCORES = 8
P = 128
F_IN = 128
H1 = 2
C1 = 64
HC1 = 128
C2 = 64
NS = 12544          # nodes per core shard = 98 * 128
NSLOT = NS // P     # 98 dst slots per core
NPAD = NS * NCORES  # 100352
NBLK = NPAD // P    # 784
W1R = 144           # layer-1 row: h(128) u(2) v(2) f(2) g(2) pad(8)
GW1 = 4 * W1R       # 576 floats per 4-row group (%64 floats)
EW1 = 192           # gather elem floats (>=136, %64)
W2R = 64            # layer-2 row: h2 only (u2/v2 computed on-chip)
GW2 = 4 * W2R       # 256 (%64)
EW2 = 64            # %64
GRP = NS // 4 + 2   # groups per core shard incl. 2 zero pad groups
NG = NCORES * GRP   # full table groups (AllGather of 8 shards)
NEG_SLOPE = 0.2


def _ensure_axon_hooks():
    """bass_utils' trace path needs antenv.axon_hooks; provide it if absent."""
    import types
    try:
        import antenv.axon_hooks as mod
    except ImportError:
        import antenv
        mod = types.ModuleType("antenv.axon_hooks")
        mod._hook = None
        def set_axon_ntff_profile_hook(hook):
            mod._hook = hook
        def get_axon_ntff_profile_hook():
            return mod._hook
        mod.set_axon_ntff_profile_hook = set_axon_ntff_profile_hook
        mod.get_axon_ntff_profile_hook = get_axon_ntff_profile_hook
        sys.modules["antenv.axon_hooks"] = mod
        antenv.axon_hooks = mod
    if mod.get_axon_ntff_profile_hook() is None:
        try:
            from trn_agent_boot.trn_boot import _ntff_profile_via_ctypes
            hook = _ntff_profile_via_ctypes("/opt/axon/libaxon_pjrt.so")
            if hook is not None:
                mod.set_axon_ntff_profile_hook(hook)
        except Exception:
            pass


# ---------------- host-side graph preprocessing ----------------------------
def _color_and_permute(src, dst):
    """Greedy 4-coloring (balance each dst's sources across classes) and a
    node permutation: pid % 4 == class, blocks sorted by class-count vector.
    Returns perm (node -> pid)."""
    odeg = np.bincount(src, minlength=NPAD)
    proc = np.argsort(-odeg, kind="stable")
    es = np.argsort(src, kind="stable")
    ss, dd = src[es], dst[es]
    starts = np.searchsorted(ss, np.arange(NPAD + 1))
    cnt = np.zeros((NPAD, 4), np.int32)     # per-dst class counts
    quota = np.full(4, NPAD // 4, np.int64)
    color = np.full(NPAD, -1, np.int8)
    has_out = odeg > 0
    for n in proc:
        if not has_out[n]:
            break  # proc is sorted by out-degree desc
        a, b = starts[n], starts[n + 1]
        nb = dd[a:b]
        score = cnt[nb].sum(0).astype(np.float64)
        score += (1.0 - quota / (NPAD // 4)) * 0.5
        score[quota <= 0] = np.inf
        c = int(np.argmin(score))
        color[n] = c
        quota[c] -= 1
        cnt[nb, c] += 1
    # refinement: move a node's color where it most reduces its dsts'
    # class-count imbalance
    for _ in range(5):
        for n in np.flatnonzero(has_out):
            a, b = starts[n], starts[n + 1]
            nb = dd[a:b]
            c1 = color[n]
            s = cnt[nb].sum(0)
            c2 = int(np.argmin(s))
            if c2 != c1 and s[c2] + len(nb) < s[c1]:
                cnt[nb, c1] -= 1
                cnt[nb, c2] += 1
                color[n] = c2
    sizes = np.bincount(color[color >= 0], minlength=4)
    while sizes.max() > NPAD // 4:
        c1 = int(sizes.argmax())
        cand = np.flatnonzero(color == c1)
        n = cand[np.argmin(odeg[cand])]
        c2 = int(sizes.argmin())
        a, b = starts[n], starts[n + 1]
        cnt[dd[a:b], c1] -= 1
        cnt[dd[a:b], c2] += 1
        color[n] = c2
        sizes[c1] -= 1
        sizes[c2] += 1
    left = np.flatnonzero(color < 0)
    fill = np.repeat(np.arange(4), (NPAD // 4 - sizes).clip(0))[:len(left)]
    color[left] = fill
    assert (np.bincount(color, minlength=4) == NPAD // 4).all()
    mx = cnt.max(1)
    am = cnt.argmax(1)
    key = np.lexsort((cnt[:, 3], cnt[:, 2], cnt[:, 1], cnt[:, 0], am, mx))
    # sorted block rank r -> (core r%8, slot r//8); within a block position
    # p takes color p%4 (pid%4 == p%4).
    queues = [key[color[key] == c] for c in range(4)]
    pos = [0, 0, 0, 0]
    perm = np.empty(NPAD, np.int64)   # node -> pid
    for g in range(NPAD):
        r, p = g // P, g % P
        c = p % 4
        n = queues[c][pos[c]]
        pos[c] += 1
        perm[n] = (r % NCORES) * NS + (r // NCORES) * P + p
    return perm


def _build_slot_tables(src, dst, perm):
    """Per-core, per-slot, per-class edge slot tables in pid space.
    K_sched[s][c] = max over the 8 cores of the block max per-partition
    class count. Returns K_sched [98][4] and per-core int16 idx streams."""
    psrc = perm[src]
    pdst = perm[dst]
    blk = pdst // P
    part = pdst % P
    cls = psrc % 4
    cnt = np.zeros((NPAD, 4), np.int32)
    np.add.at(cnt, (pdst, cls), 1)
    bmax = cnt.reshape(NBLK, P, 4).max(1)             # [784, 4]
    # pid block b: core b % 8 (rank r=b? no: pid = (r%8)*NS + (r//8)*128+p)
    # => block index in pid space: b = core*NSLOT + slot
    K_sched = bmax.reshape(NBLK // NSLOT, NSLOT, 4).max(0)  # wrong axis fix below
    # recompute correctly: pid block b: core b // NSLOT, slot b % NSLOT
    K_sched = bmax.reshape(NCORES, NSLOT, 4).max(0)   # [98, 4]
    ktot = K_sched.sum(1)

    order = np.lexsort((part, cls, blk))
    pb, pc, pp, ps = blk[order], cls[order], part[order], psrc[order]
    key = (pb * 4 + pc) * P + pp
    counts = np.bincount(key, minlength=NBLK * 4 * P)
    kpos = np.arange(len(order)) - np.repeat(
        np.concatenate([[0], np.cumsum(counts)[:-1]]), counts)

    # idx value = table group row: shards AllGathered with 2 zero pad groups
    # per core appended, so group g lands at row g + 2*(g//(NS//4));
    # pad slots point at core 0's zero rows (row NS//4).
    PADG = NS // 4
    koff = np.zeros((NSLOT, 4), np.int64)
    koff[:, 1:] = np.cumsum(K_sched, 1)[:, :-1]
    soff = np.concatenate([[0], np.cumsum(ktot)])
    total_cols = int(soff[-1])
    idx = np.full((NCORES, total_cols * P), PADG, np.int32)
    core = pb // NSLOT
    slot = pb % NSLOT
    pos_in_stream = (soff[slot] + koff[slot, pc] + kpos) * P + pp
    grp = ps // 4
    idx[core, pos_in_stream] = grp + 2 * (grp // (NS // 4))
    # wrap in 16 partitions: position i -> [i%16, i//16]; replicate 8x.
    idx16 = np.ascontiguousarray(
        idx.reshape(NCORES, total_cols * P // 16, 16).transpose(0, 2, 1)
    ).astype(np.int16)
    idx16 = np.ascontiguousarray(np.tile(idx16, (1, 8, 1)))
    return K_sched, idx16, soff


def _att_cat(W, att_src, att_dst):
    h, c = att_src.shape
    cin = W.shape[1]
    As = np.zeros((cin, h), np.float32)
    Ad = np.zeros((cin, h), np.float32)
    for i in range(h):
        As[i * c:(i + 1) * c, i] = att_src[i]
        Ad[i * c:(i + 1) * c, i] = att_dst[i]
    return np.concatenate([W, W @ As, W @ Ad], 1).astype(np.float32)


# ---------------- bass program --------------------------------------------
def _build_program(K_sched, soff):
    import concourse.bass as bass
    import concourse.tile as tile
    from concourse import mybir, library_config
    from concourse.library_overlay import lower_extended_insts
    from concourse.vector_clock import ScopedClock

    f32 = mybir.dt.float32
    i16 = mybir.dt.int16
    Act = mybir.ActivationFunctionType
    Alu = mybir.AluOpType
    X = mybir.AxisListType.X

    total_cols = int(soff[-1])
    kmax = int(max(K_sched.sum(1)))

    class PatchedTileContext(tile.TileContext):
        """Kernel-tail drain must not carry more waits than the ISA allows;
        split them across chained drains (this walrus allows 1 wait/inst)."""
        def _drain_and_barrier(self, tick_clock, wait_clock):
            drain_inst = self.nc.sync.drain()
            wait_clock.add_sem_waits(
                drain_inst.ins, ScopedClock({None: tick_clock.global_clock})
            )
            si = drain_inst.ins.sync_info
            if si is not None and si.on_wait and len(si.on_wait) > 1:
                waits = list(si.on_wait)
                si.on_wait = waits[:1]
                rest = waits[1:]
                while rest:
                    extra = self.nc.sync.drain()
                    extra.ins.sync_info = mybir.SyncInfo(on_wait=rest[:1], on_update=[])
                    rest = rest[1:]
            self.nc.all_engine_barrier()
            assert self.sems is not None
            popped = self.nc._tile_sem_poison_stack.pop()
            assert popped is self._sem_poison
            self.nc.clear_and_free_semaphores(list(self.sems.allocated().values()))
            self.nc.all_engine_barrier()

    nc = bass.Bass(num_devices=NCORES, num_swdge_queues=4)

    xTs = nc.declare_dram_parameter("xTs", [P, NS], f32, isOutput=False)
    w1cat = nc.declare_dram_parameter("w1cat", [P, HC1 + 2 * H1], f32, isOutput=False)
    w2cat = nc.declare_dram_parameter("w2cat", [P, C2 + 2], f32, isOutput=False)
    b1row = nc.declare_dram_parameter("b1row", [1, HC1], f32, isOutput=False)
    b2row = nc.declare_dram_parameter("b2row", [1, C2], f32, isOutput=False)
    as2row = nc.declare_dram_parameter("as2row", [1, C2], f32, isOutput=False)
    ident_in = nc.declare_dram_parameter("ident", [P, P], f32, isOutput=False)
    idx_in = nc.declare_dram_parameter("idx16", [P, total_cols * P // 16],
                                       i16, isOutput=False)
    padg1 = nc.declare_dram_parameter("padg1", [2, GW1], f32, isOutput=False)
    padg2 = nc.declare_dram_parameter("padg2", [2, GW2], f32, isOutput=False)
    out2 = nc.declare_dram_parameter("out2", [NS, C2], f32, isOutput=True)

    with PatchedTileContext(nc) as tc, ExitStack() as ctx:
        nc.gpsimd.load_library(library_config.mlp)
        const = ctx.enter_context(tc.tile_pool(name="const", bufs=1))
        dram = ctx.enter_context(tc.tile_pool(name="dram", bufs=1, space="DRAM"))

        tab1s = dram.tile([GRP, GW1], f32)                     # own shard L1
        tab1 = dram.tile([NG, GW1], f32, addr_space="Shared")  # full L1
        tab2s = dram.tile([GRP, GW2], f32)
        tab2 = dram.tile([NG, GW2], f32, addr_space="Shared")

        w1_sb = const.tile([P, HC1 + 2 * H1], f32)
        nc.sync.dma_start(out=w1_sb[:], in_=w1cat[:])
        w2_sb = const.tile([P, C2 + 2], f32)
        nc.sync.dma_start(out=w2_sb[:], in_=w2cat[:])
        b1_sb = const.tile([P, HC1], f32)
        nc.sync.dma_start(out=b1_sb[:], in_=b1row[0:1, :].to_broadcast([P, HC1]))
        b2_sb = const.tile([P, C2], f32)
        nc.sync.dma_start(out=b2_sb[:], in_=b2row[0:1, :].to_broadcast([P, C2]))
        as2_sb = const.tile([P, C2], f32)
        nc.sync.dma_start(out=as2_sb[:], in_=as2row[0:1, :].to_broadcast([P, C2]))
        ident_sb = const.tile([P, P], f32)
        nc.sync.dma_start(out=ident_sb[:], in_=ident_in[:])
        idx_sb = const.tile([P, total_cols * P // 16], i16)
        nc.sync.dma_start(out=idx_sb[:], in_=idx_in[:])
        uvfg2 = const.tile([P, NSLOT * 3], f32)   # per-slot [selfw2, f2, g2]
        sw1 = const.tile([P, NSLOT * 2], f32)     # per-node layer-1 self weight
        # zero pad groups at each shard's tail (u=v=0 -> pad slots add 0)
        nc.sync.dma_start(out=tab1s[GRP - 2:GRP, :], in_=padg1[:])
        nc.sync.dma_start(out=tab2s[GRP - 2:GRP, :], in_=padg2[:])

        # ---- p1: own-shard node transform -> tab1s ----
        with nc.named_scope("p1"), ExitStack() as c2:
            sbp = c2.enter_context(tc.tile_pool(name="p1sb", bufs=3))
            psp = c2.enter_context(tc.tile_pool(name="p1ps", bufs=3, space="PSUM"))
            for sl in range(14):
                slab = sbp.tile([P, 7 * P], f32, tag="slab")
                nc.sync.dma_start(out=slab[:], in_=xTs[:, sl * 896:(sl + 1) * 896])
                for k in range(7):
                    s = sl * 7 + k
                    ps = psp.tile([P, HC1 + 2 * H1], f32, tag="ps")
                    nc.tensor.matmul(out=ps[:], lhsT=slab[:, k * P:(k + 1) * P],
                                     rhs=w1_sb[:], start=True, stop=True)
                    t1 = sbp.tile([P, W1R], f32, tag="t1")
                    nc.scalar.copy(out=t1[:, 0:HC1], in_=ps[:, 0:HC1])
                    nc.scalar.activation(out=t1[:, 128:130],
                                         in_=ps[:, 128:130], func=Act.Exp)
                    nc.scalar.activation(out=t1[:, 130:132],
                                         in_=ps[:, 128:130], func=Act.Exp,
                                         scale=NEG_SLOPE)
                    nc.scalar.activation(out=t1[:, 132:134],
                                         in_=ps[:, 130:132], func=Act.Exp)
                    nc.scalar.activation(out=t1[:, 134:136],
                                         in_=ps[:, 130:132], func=Act.Exp,
                                         scale=NEG_SLOPE)
                    nc.vector.memset(t1[:, 136:W1R], 0.0)
                    sa1 = sbp.tile([P, 4], f32, tag="sa1")
                    nc.vector.tensor_tensor(out=sa1[:, 0:2], in0=t1[:, 128:130],
                                            in1=t1[:, 132:134], op=Alu.mult)
                    nc.vector.tensor_tensor(out=sa1[:, 2:4], in0=t1[:, 130:132],
                                            in1=t1[:, 134:136], op=Alu.mult)
                    nc.vector.tensor_tensor(out=sw1[:, 2 * s:2 * s + 2],
                                            in0=sa1[:, 0:2], in1=sa1[:, 2:4],
                                            op=Alu.max)
                    nc.sync.dma_start(
                        out=tab1s[s * 32:(s + 1) * 32, :].rearrange(
                            "g (r w) -> (g r) w", r=4),
                        in_=t1[:])

        # ---- AllGather layer-1 table ----
        nc.gpsimd.collective_compute(
            "AllGather", mybir.AluOpType.bypass,
            replica_groups=[list(range(NCORES))],
            ins=[tab1s[:, :].opt()],
            outs=[tab1[:, :].opt()],
        )

        reg_cache = {}
        qctr = [0]

        def nreg(n):
            if n not in reg_cache:
                reg_cache[n] = nc.gpsimd.to_reg(n)
            return reg_cache[n]

        def issue_gather(pool, s, tab, gw, ew, w_row):
            ktot = int(K_sched[s].sum())
            g = pool.tile([P, kmax * ew], f32, tag="g")
            co = 0
            for c in range(4):
                kc = int(K_sched[s][c])
                k0 = 0
                while k0 < kc:
                    kch = min(kc - k0, 8)   # <=1024 descriptors per call
                    n_idx = P * kch
                    ioff = (int(soff[s]) + co) * P // 16
                    in_ap = bass.AP(tab[:, :].tensor, c * w_row,
                                    [[gw, NG - 1], [1, ew]])
                    nc.gpsimd.dma_gather(
                        out_ap=g[:, co * ew:(co + kch) * ew].rearrange(
                            "p (k e) -> p k e", k=kch),
                        in_ap=in_ap,
                        idxs_ap=idx_sb[:, ioff:ioff + n_idx // 16],
                        num_idxs=n_idx, num_idxs_reg=nreg(n_idx),
                        elem_size=ew, elem_step=gw,
                        queue_num=qctr[0] % 4,
                    )
                    qctr[0] += 1
                    co += kch
                    k0 += kch
            return g, ktot

        # ---- e1: layer-1 edge phase (emits layer-2 table rows) ----
        with nc.named_scope("e1"), ExitStack() as c2:
            sbg = c2.enter_context(tc.tile_pool(name="e1g", bufs=3))
            sbo = c2.enter_context(tc.tile_pool(name="e1o", bufs=4))
            sbm = c2.enter_context(tc.tile_pool(name="e1m", bufs=2))
            sbs = c2.enter_context(tc.tile_pool(name="e1s", bufs=3))
            psp = c2.enter_context(tc.tile_pool(name="e1ps", bufs=2, space="PSUM"))

            def issue1(s):
                g, ktot = issue_gather(sbg, s, tab1, GW1, EW1, W1R)
                ow = sbo.tile([P, W1R], f32, tag="ow")
                nc.sync.dma_start(
                    out=ow[:],
                    in_=tab1s[s * 32:(s + 1) * 32, :].rearrange(
                        "g (r w) -> (g r) w", r=4))
                return g, ktot, ow

            PF = 3
            pre = [issue1(s) for s in range(PF)]
            for s in range(NSLOT):
                g, K, ow = pre.pop(0)
                if s + PF < NSLOT:
                    pre.append(issue1(s + PF))
                gv = g[:, 0:K * EW1].rearrange("p (k w) -> p k w", w=EW1)
                # per-edge uf/vg for both heads: [P, K, 2]
                uf2 = sbs.tile([P, 2 * kmax], f32, tag="uf2")
                nc.vector.tensor_tensor(
                    out=uf2[:, 0:2 * K].rearrange("p (k h) -> p k h", h=2),
                    in0=gv[:, :, 128:130],
                    in1=ow[:, 132:134].rearrange("p h -> p () h").to_broadcast(
                        [P, K, 2]),
                    op=Alu.mult)
                vg2 = sbs.tile([P, 2 * kmax], f32, tag="vg2")
                nc.vector.tensor_tensor(
                    out=vg2[:, 0:2 * K].rearrange("p (k h) -> p k h", h=2),
                    in0=gv[:, :, 130:132],
                    in1=ow[:, 134:136].rearrange("p h -> p () h").to_broadcast(
                        [P, K, 2]),
                    op=Alu.mult)
                selfw = sw1[:, 2 * s:2 * s + 2]
                # w = max(uf, vg); den = self_w + sum_k w
                wt2 = sbs.tile([P, 2 * kmax], f32, tag="wt2")
                nc.vector.tensor_tensor(out=wt2[:, 0:2 * K], in0=uf2[:, 0:2 * K],
                                        in1=vg2[:, 0:2 * K], op=Alu.max)
                den0 = sbs.tile([P, 2], f32, tag="den0")
                nc.vector.tensor_reduce(
                    out=den0[:],
                    in_=wt2[:, 0:2 * K].rearrange("p (k h) -> p h k", h=2),
                    axis=X, op=Alu.add)
                den = sbs.tile([P, 2], f32, tag="den")
                nc.vector.tensor_tensor(out=den[:], in0=den0[:], in1=selfw,
                                        op=Alu.add)
                # messages and numerator (both heads in one op each)
                msgs = sbm.tile([P, kmax * HC1], f32, tag="msgs")
                nc.vector.tensor_tensor(
                    out=msgs[:, 0:K * HC1].rearrange(
                        "p (k h c) -> p k h c", h=2, c=C1),
                    in0=gv[:, :, 0:HC1].rearrange(
                        "p k (h c) -> p k h c", c=C1),
                    in1=wt2[:, 0:2 * K].rearrange(
                        "p (k h) -> p k h ()", h=2).to_broadcast([P, K, 2, C1]),
                    op=Alu.mult)
                numr = sbs.tile([P, HC1], f32, tag="numr")
                nc.vector.tensor_reduce(
                    out=numr[:],
                    in_=msgs[:, 0:K * HC1].rearrange("p (k c) -> p c k", c=HC1),
                    axis=X, op=Alu.add)
                rec = sbs.tile([P, 2], f32, tag="rec")
                nc.vector.reciprocal(out=rec[:], in_=den[:])
                ob = sbs.tile([P, HC1], f32, tag="ob")
                for h in range(2):
                    cs = slice(h * C1, (h + 1) * C1)
                    num3 = sbs.tile([P, C1], f32, tag=f"num3{h}")
                    nc.vector.scalar_tensor_tensor(
                        out=num3[:], in0=ow[:, cs], scalar=sw1[:, 2 * s + h:2 * s + h + 1],
                        in1=numr[:, cs], op0=Alu.mult, op1=Alu.add)
                    nc.vector.scalar_tensor_tensor(
                        out=ob[:, cs], in0=num3[:], scalar=rec[:, h:h + 1],
                        in1=b1_sb[:, cs], op0=Alu.mult, op1=Alu.add)
                # ELU -> transpose -> @W2cat -> layer-2 table row
                negm = sbs.tile([P, HC1], f32, tag="negm")
                nc.vector.tensor_scalar_min(out=negm[:], in0=ob[:], scalar1=0.0)
                pos = sbs.tile([P, HC1], f32, tag="pos")
                nc.scalar.activation(out=pos[:], in_=ob[:], func=Act.Relu)
                em = sbs.tile([P, HC1], f32, tag="em")
                nc.scalar.activation(out=em[:], in_=negm[:], func=Act.Exp)
                h1b = sbs.tile([P, HC1], f32, tag="h1b")
                nc.vector.scalar_tensor_tensor(
                    out=h1b[:], in0=em[:], scalar=-1.0, in1=pos[:],
                    op0=Alu.add, op1=Alu.add)
                ps_t = psp.tile([P, P], f32, tag="pst")
                nc.tensor.transpose(out=ps_t[:], in_=h1b[:], identity=ident_sb[:])
                h1t = sbs.tile([P, P], f32, tag="h1t")
                nc.scalar.copy(out=h1t[:], in_=ps_t[:])
                ps2 = psp.tile([P, C2 + 2], f32, tag="ps2")
                nc.tensor.matmul(out=ps2[:], lhsT=h1t[:], rhs=w2_sb[:],
                                 start=True, stop=True)
                t2 = sbs.tile([P, W2R], f32, tag="t2")
                nc.scalar.copy(out=t2[:, 0:C2], in_=ps2[:, 0:C2])
                nc.scalar.activation(out=uvfg2[:, 3 * s + 1:3 * s + 2],
                                     in_=ps2[:, C2 + 1:C2 + 2], func=Act.Exp)
                nc.scalar.activation(out=uvfg2[:, 3 * s + 2:3 * s + 3],
                                     in_=ps2[:, C2 + 1:C2 + 2], func=Act.Exp,
                                     scale=NEG_SLOPE)
                se2 = sbs.tile([P, 4], f32, tag="se2")
                nc.scalar.activation(out=se2[:, 0:1], in_=ps2[:, C2:C2 + 1],
                                     func=Act.Exp)
                nc.scalar.activation(out=se2[:, 1:2], in_=ps2[:, C2:C2 + 1],
                                     func=Act.Exp, scale=NEG_SLOPE)
                sm2 = sbs.tile([P, 2], f32, tag="sm2")
                nc.vector.tensor_tensor(out=sm2[:, 0:1], in0=se2[:, 0:1],
                                        in1=uvfg2[:, 3 * s + 1:3 * s + 2],
                                        op=Alu.mult)
                nc.vector.tensor_tensor(out=sm2[:, 1:2], in0=se2[:, 1:2],
                                        in1=uvfg2[:, 3 * s + 2:3 * s + 3],
                                        op=Alu.mult)
                nc.vector.tensor_tensor(out=uvfg2[:, 3 * s:3 * s + 1],
                                        in0=sm2[:, 0:1], in1=sm2[:, 1:2],
                                        op=Alu.max)
                nc.sync.dma_start(
                    out=tab2s[s * 32:(s + 1) * 32, :].rearrange(
                        "g (r w) -> (g r) w", r=4),
                    in_=t2[:])

        # ---- AllGather layer-2 table ----
        nc.gpsimd.collective_compute(
            "AllGather", mybir.AluOpType.bypass,
            replica_groups=[list(range(NCORES))],
            ins=[tab2s[:, :].opt()],
            outs=[tab2[:, :].opt()],
        )

        # ---- e2: layer-2 edge phase -> out2 ----
        with nc.named_scope("e2"), ExitStack() as c2:
            sbg = c2.enter_context(tc.tile_pool(name="e2g", bufs=7))
            sbo = c2.enter_context(tc.tile_pool(name="e2o", bufs=8))
            sbm = c2.enter_context(tc.tile_pool(name="e2m", bufs=2))
            sbs = c2.enter_context(tc.tile_pool(name="e2s", bufs=3))

            def issue2(s):
                g, ktot = issue_gather(sbg, s, tab2, GW2, EW2, W2R)
                ow = sbo.tile([P, W2R], f32, tag="ow")
                nc.sync.dma_start(
                    out=ow[:],
                    in_=tab2s[s * 32:(s + 1) * 32, :].rearrange(
                        "g (r w) -> (g r) w", r=4))
                return g, ktot, ow

            PF = 6
            pre = [issue2(s) for s in range(PF)]
            for s in range(NSLOT):
                g, K, ow = pre.pop(0)
                if s + PF < NSLOT:
                    pre.append(issue2(s + PF))
                gv = g[:, 0:K * EW2].rearrange("p (k w) -> p k w", w=EW2)
                # per-edge a_s2 dot, then u2/v2 via exp
                prod = sbm.tile([P, kmax * C2], f32, tag="prod")
                nc.vector.tensor_tensor(
                    out=prod[:, 0:K * C2].rearrange("p (k c) -> p k c", c=C2),
                    in0=gv[:, :, 0:C2],
                    in1=as2_sb[:].rearrange("p c -> p () c").to_broadcast(
                        [P, K, C2]),
                    op=Alu.mult)
                tdot = sbs.tile([P, kmax], f32, tag="tdot")
                nc.vector.tensor_reduce(
                    out=tdot[:, 0:K],
                    in_=prod[:, 0:K * C2].rearrange("p (k c) -> p k c", k=K),
                    axis=X, op=Alu.add)
                u2 = sbs.tile([P, kmax], f32, tag="u2")
                nc.scalar.activation(out=u2[:, 0:K], in_=tdot[:, 0:K],
                                     func=Act.Exp)
                v2 = sbs.tile([P, kmax], f32, tag="v2")
                nc.scalar.activation(out=v2[:, 0:K], in_=tdot[:, 0:K],
                                     func=Act.Exp, scale=NEG_SLOPE)
                uf = sbs.tile([P, kmax], f32, tag="uf")
                nc.vector.tensor_scalar_mul(
                    out=uf[:, 0:K], in0=u2[:, 0:K],
                    scalar1=uvfg2[:, 3 * s + 1:3 * s + 2])
                selfw = uvfg2[:, 3 * s:3 * s + 1]
                wt = sbs.tile([P, kmax], f32, tag="wt")
                nc.vector.scalar_tensor_tensor(
                    out=wt[:, 0:K], in0=v2[:, 0:K],
                    scalar=uvfg2[:, 3 * s + 2:3 * s + 3],
                    in1=uf[:, 0:K], op0=Alu.mult, op1=Alu.max)
                den0 = sbs.tile([P, 1], f32, tag="den0")
                nc.vector.tensor_reduce(out=den0[:], in_=wt[:, 0:K],
                                        axis=X, op=Alu.add)
                den = sbs.tile([P, 1], f32, tag="den")
                nc.vector.tensor_tensor(out=den[:], in0=den0[:], in1=selfw,
                                        op=Alu.add)
                msgs = sbm.tile([P, kmax * C2], f32, tag="msgs")
                nc.vector.tensor_tensor(
                    out=msgs[:, 0:K * C2].rearrange("p (k c) -> p k c", c=C2),
                    in0=gv[:, :, 0:C2],
                    in1=wt[:, 0:K].rearrange("p k -> p k ()").to_broadcast(
                        [P, K, C2]),
                    op=Alu.mult)
                numr = sbs.tile([P, C2], f32, tag="numr")
                nc.vector.tensor_reduce(
                    out=numr[:],
                    in_=msgs[:, 0:K * C2].rearrange("p (k c) -> p c k", c=C2),
                    axis=X, op=Alu.add)
                rec = sbs.tile([P, 1], f32, tag="rec")
                nc.vector.reciprocal(out=rec[:], in_=den[:])
                num3 = sbs.tile([P, C2], f32, tag="num3")
                nc.vector.scalar_tensor_tensor(
                    out=num3[:], in0=ow[:, 0:C2], scalar=selfw,
                    in1=numr[:], op0=Alu.mult, op1=Alu.add)
                ob = sbs.tile([P, C2], f32, tag="ob")
                nc.vector.scalar_tensor_tensor(
                    out=ob[:], in0=num3[:], scalar=rec[:, 0:1],
                    in1=b2_sb[:], op0=Alu.mult, op1=Alu.add)
                nc.sync.dma_start(out=out2[s * P:(s + 1) * P, :], in_=ob[:])

    _split_overloaded_waits(nc)
    lower_extended_insts(nc)
    return nc


def _split_overloaded_waits(nc):
    """This walrus build accepts one sem wait per instruction; hoist extras
    onto NoOps spliced immediately before (same engine => same ordering)."""
    from concourse import mybir
    n_fix = 0
    for bb in nc.main_func.blocks:
        insts = bb.instructions
        out = []
        for ins in insts:
            si = getattr(ins, "sync_info", None)
            waits = list(si.on_wait) if (si and si.on_wait) else []
            if len(waits) > 1:
                si.on_wait = waits[-1:]
                rest = waits[:-1]
                while rest:
                    nop = mybir.InstNoOp(name=f"wsplit-{nc.next_id()}", ins=[], outs=[])
                    nop.engine = ins.engine
                    nop.sync_info = mybir.SyncInfo(on_wait=rest[:1], on_update=[])
                    rest = rest[1:]
                    out.append(nop)
                n_fix += 1
            out.append(ins)
        if len(out) != len(insts):
            insts.clear()
            insts.extend(out)
    return n_fix


# ---------------- entry point ----------------------------------------------
_LAST_EXEC_NS = None
_LAST_SCOPES = None


def kernel(x, edge_index, W1, att_src1, att_dst1, b1, W2, att_src2, att_dst2,
           b2, _trace=False):
    global _LAST_EXEC_NS, _LAST_SCOPES
    W1m, W2m = W1, W2
    _ensure_axon_hooks()
    import concourse.bass_utils as bass_utils
    bass_utils.upload_artifacts = lambda tmpdir: tmpdir
    from concourse.bass_utils import run_bass_kernel_spmd

    x = np.asarray(x, np.float32)
    src = np.asarray(edge_index[0], np.int64)
    dst = np.asarray(edge_index[1], np.int64)

    perm = _color_and_permute(src, dst)
    K_sched, idx16, soff = _build_slot_tables(src, dst, perm)

    w1c = _att_cat(np.asarray(W1m, np.float32),
                   np.asarray(att_src1, np.float32),
                   np.asarray(att_dst1, np.float32))
    w2c = _att_cat(np.asarray(W2m, np.float32),
                   np.asarray(att_src2, np.float32),
                   np.asarray(att_dst2, np.float32))
    b1r = np.asarray(b1, np.float32).reshape(1, HC1)
    b2r = np.asarray(b2, np.float32).reshape(1, C2)

    inv = np.empty(NPAD, np.int64)   # pid -> node
    inv[perm] = np.arange(NPAD)
    xp = np.zeros((NPAD, F_IN), np.float32)
    real = inv < N
    xp[real] = x[inv[real]]
    ident = np.eye(P, dtype=np.float32)
    # pad rows: h2 dot att_src2 -> -inf so exp()=0 (pad slots add nothing)
    as2vec = np.asarray(att_src2, np.float32).reshape(C2)
    padrow2 = (-1e18 * np.sign(as2vec)).astype(np.float32)
    padg2 = np.tile(padrow2, (2, 4)).astype(np.float32)

    nc = _build_program(K_sched, soff)
    in_maps = []
    for d in range(NCORES):
        xTs = np.ascontiguousarray(xp[d * NS:(d + 1) * NS].T)
        in_maps.append(dict(
            xTs=xTs, w1cat=w1c, w2cat=w2c, b1row=b1r, b2row=b2r,
            as2row=as2vec.reshape(1, C2),
            ident=ident, idx16=np.ascontiguousarray(idx16[d]),
            padg1=np.zeros((2, GW1), np.float32),
            padg2=padg2,
        ))
    res = run_bass_kernel_spmd(nc, in_maps, list(range(NCORES)), trace=_trace)
    _LAST_EXEC_NS = res.exec_time_ns
    _LAST_SCOPES = res.per_core_scope_times
    outp = np.concatenate([res.results[d]["out2"] for d in range(NCORES)], 0)
    out = np.empty((N, C2), np.float32)
    out[:] = outp[perm[:N]]
    return out
